# revision 15
# baseline (speedup 1.0000x reference)
"""Trainium2 Bass kernel for nn_Block1_54279796687228 (retrieval_knn).

Math: the reference builds the full per-sample Jacobian J of the conv
encoder and contracts it with x.  For a conv+ReLU (piecewise-linear)
encoder, einsum(x, J) is exactly the JVP of the encoder at x in
direction x:

    z_q = m2 * conv2_nobias(m1 * conv1_nobias(x)),
    m1 = [conv1(x)+b1 > 0],  m2 = [conv2(relu(conv1(x)+b1))+b2 > 0]

With the zero biases produced by setup_inputs() this collapses to the
plain forward pass relu(conv2(relu(conv1(x)))).  Both variants are
implemented; the host picks based on the actual bias values.

Lowering (no-bias fast path):
  conv1 -> one K=48 matmul over a host-built im2col (layout only);
           im2col and w1r ride in ONE packed DMA.
  conv2 -> fold (ci,kw) into K=128: the ReLU+shift+f32r cast fuse into
           4 windowed vector ops straight out of PSUM; then 4
           accumulating matmuls (one per kh).
  Hopfield -> scores are computed directly TRANSPOSED, (mem, pos), as
           4 matmuls with lkT chunks stationary -- no softmax-axis
           transpose is ever needed.  The exp is split four ways over
           two alternating PSUM banks so each G matmul chases its own
           exp chunk while the PE streams.  The lookup chunks arrive
           in natural layout with an appended ones-column (layout only),
           so the 4 accumulating G matmuls produce [G; Z] in one go
           (Z = softmax denominator) and no on-device transposes of the
           lookup are needed.  The final projection matmul uses a
           [Wvo | e_Z | 0] operand (border pre-staged from the host,
           Wv@Wo folded on device off the critical path): its output
           column 64 IS Z^T, so no separate Z-transpose matmul exists
           (the zero pad column keeps the fp32r even-extent rule).
           out2 = (G.T @ [Wvo|e])[:, :64] / Z, emitted (pos, ch'); the
           host transposes each (64,64) sample for free.  The output is
           scaled and stored in two staggered halves (vector + scalar
           engines, separate tiles) feeding two DMA queues.

All matmuls run in float32r (single pass); ~3e-4 relative error
end-to-end vs the fp32 reference.

Sharding: pure data parallel over batch. Sample b runs on cores b and
b+4 (duplicates); host gathers from cores 0-3.
"""

import os
import numpy as np

# -- NTFF profile hook shim -------------------------------------------------
# bass_utils' trace path needs antenv.axon_hooks, which this image's antenv
# lacks. Register the ctypes-based hook from trn_agent_boot if available so
# trace=True / BASS_TRACE=1 works; degrade silently otherwise.
def _ensure_ntff_hook():
    try:
        import antenv.axon_hooks  # noqa: F401
        return
    except ImportError:
        pass
    try:
        import sys, types
        import antenv
        from trn_agent_boot.trn_boot import _ntff_profile_via_ctypes

        mod = types.ModuleType("antenv.axon_hooks")
        _h = [None]
        mod.set_axon_ntff_profile_hook = lambda h: _h.__setitem__(0, h)
        mod.get_axon_ntff_profile_hook = lambda: _h[0]
        sys.modules["antenv.axon_hooks"] = mod
        antenv.axon_hooks = mod
        so = "/opt/axon/libaxon_pjrt.so"
        if os.path.exists(so):
            mod.set_axon_ntff_profile_hook(_ntff_profile_via_ctypes(so))
    except Exception:
        pass


_ensure_ntff_hook()

import concourse.bacc as bacc
import concourse.bass as bass
import concourse.tile as tile
import concourse.mybir as mybir
from concourse.bass_utils import run_bass_kernel_spmd

F32 = mybir.dt.float32
F32R = mybir.dt.float32r
RELU = mybir.ActivationFunctionType.Relu
EXP = mybir.ActivationFunctionType.Exp

B, CIN, C1, C2, Q = 4, 3, 32, 64, 512  # batch, in-ch, conv1-ch, conv2-ch, memories
N_CORES = 8

_COMPILED = {}  # variant -> nc
last_exec_time_ns = None
last_trace_path = None


def _build_fast():
    """No-bias variant: forward pass + Hopfield, latency-optimized."""
    nc = bacc.Bacc("TRN2", target_bir_lowering=False, debug=False,
                   enable_asserts=False)

    # xw: im2col of padded x (48x256) with w1r (48x32) appended -> one DMA.
    xw_d = nc.dram_tensor("xw", [48, 288], F32R, kind="ExternalInput")
    w2k_d = nc.dram_tensor("w2k", [128, 4, 64], F32R, kind="ExternalInput")
    lkT_d = nc.dram_tensor("lkT", [64, 512], F32R, kind="ExternalInput")
    # lko: lookup in natural layout, chunked (128, 4, 65) with a ones col.
    lko_d = nc.dram_tensor("lko", [128, 4, 65], F32R, kind="ExternalInput")
    # tail: [wvT | wo | wvo66-border template | pad] (65 x 256; 1KB rows).
    tail_d = nc.dram_tensor("tail", [65, 256], F32R, kind="ExternalInput")
    out_d = nc.dram_tensor("out", [64, 64], F32, kind="ExternalOutput")

    with tile.TileContext(nc) as tc:
        with (
            tc.tile_pool(name="consts", bufs=1) as consts,
            tc.tile_pool(name="work", bufs=1) as work,
            tc.tile_pool(name="psA", bufs=1, space="PSUM") as psA,
        ):
            # ---- input DMAs, one per tensor, ordered by when they gate
            # compute.  HWDGE (sync/scalar) queues land ~1.6us after
            # desc-gen vs ~2.4us for SWDGE (gpsimd), so everything that
            # can stall the PE FIFO goes on sync: xw (conv1) first, then
            # tail (the wvo matmul sits early in the PE stream), then
            # lkT.  w2k (conv2) on scalar; only lko rides SWDGE.
            sb_xw = consts.tile([48, 288], F32R, tag="xw")
            nc.sync.dma_start(sb_xw[:], xw_d.ap())
            sb_w2 = consts.tile([128, 4, 64], F32R, tag="w2")
            nc.scalar.dma_start(sb_w2[:], w2k_d.ap())
            sb_tail = consts.tile([65, 256], F32R, tag="tail")
            nc.sync.dma_start(sb_tail[:], tail_d.ap())
            sb_lkT = consts.tile([64, 512], F32R, tag="lkT")
            nc.sync.dma_start(sb_lkT[:], lkT_d.ap())
            sb_lko = consts.tile([128, 4, 65], F32R, tag="lko")
            nc.gpsimd.dma_start(sb_lko[:], lko_d.ap())

            # f32r tiles cannot be memset directly; zero imkw's pad region
            # via a cast-copy from an f32 zero tile (early, no deps).
            sb_zero = consts.tile([128, 18, 8], F32, tag="zero")
            nc.vector.memset(sb_zero[:], 0.0)
            imkw = work.tile([128, 18, 8], F32R, tag="imkw")
            nc.vector.tensor_copy(imkw[:], sb_zero[:])

            # ---- conv1: (48,32).T @ (48,256) -> (32, 16, 16) ----
            p_z1 = psA.tile([32, 16, 16], F32, tag="z1")
            nc.tensor.matmul(p_z1[:], sb_xw[:, 256:288], sb_xw[:, :256],
                             start=True, stop=True)

            # ---- conv2 input: imkw[(kw,ci), row, c] = a1pad[ci, row, 2c+kw]
            # where a1pad = zero-pad(relu(z1)).  ReLU + shift + f32r cast
            # fuse into one windowed op per kw, straight from PSUM.  All
            # four stay on the vector engine: Tile serializes same-tile
            # writers across engines anyway, and a scalar-engine shift
            # would let unrelated scalar work interleave into the chain.
            nc.vector.tensor_scalar_max(
                imkw[0:32, 1:17, 1:8], p_z1[:, :, 1:15:2], 0.0)
            nc.vector.tensor_scalar_max(
                imkw[32:64, 1:17, 0:8], p_z1[:, :, 0:16:2], 0.0)
            nc.vector.tensor_scalar_max(
                imkw[64:96, 1:17, 0:8], p_z1[:, :, 1:16:2], 0.0)
            nc.vector.tensor_scalar_max(
                imkw[96:128, 1:17, 0:7], p_z1[:, :, 2:16:2], 0.0)

            # ---- conv2: 4 accumulating matmuls (one per kh) ----
            p_z2 = psA.tile([64, 64], F32, tag="z2")
            for kh in range(4):
                nc.tensor.matmul(
                    p_z2[:],
                    sb_w2[:, kh, :],
                    imkw[:, kh:min(kh + 16, 18):2, :],
                    start=(kh == 0), stop=(kh == 3),
                )

            # ---- Wvo = Wv @ Wo; PE is otherwise idle while zq is built.
            p_wvo = psA.tile([64, 64], F32, tag="wvo")
            nc.tensor.matmul(p_wvo[:], sb_tail[0:64, 0:64],
                             sb_tail[0:64, 64:128], start=True, stop=True)

            # zq relu on the scalar engine: shorter from PSUM and keeps
            # the vector queue free for the imkw shifts.
            sb_zq = work.tile([64, 64], F32R, tag="zq")
            nc.scalar.activation(sb_zq[:], p_z2[:], RELU)

            # wvo65 = [Wvo | e_Z]: border (zeros + lone 1 at [64,64]) came
            # from the host inside tail; the copy fills the Wvo block.
            sb_wvo65 = sb_tail[0:65, 128:194]
            nc.scalar.copy(sb_wvo65[0:64, 0:64], p_wvo[:])

            # ---- scoresT: (mem128, pos) chunks; lkT chunk stationary.
            # Chunks alternate between two PSUM tiles (= two banks) so
            # exp of chunk c can run while the PE writes chunk c+1: the
            # exp is split four ways and each G matmul chases its own
            # exp chunk.
            p_sT0 = psA.tile([128, 2, 64], F32, tag="sT0")
            p_sT1 = psA.tile([128, 2, 64], F32, tag="sT1")
            sT = lambda c: (p_sT0 if c % 2 == 0 else p_sT1)[:, c // 2, :]
            for c in range(4):
                nc.tensor.matmul(
                    sT(c),
                    sb_lkT[:, 128 * c:128 * (c + 1)], sb_zq[:],
                    start=True, stop=True,
                )

            # unnormalized softmax: E = exp(s/8).  |s/8| << 1 here, so
            # max-subtraction is unnecessary in fp32.
            sb_E0 = work.tile([128, 2, 64], F32R, tag="E0")
            sb_E1 = work.tile([128, 2, 64], F32R, tag="E1")
            E = lambda c: (sb_E0 if c % 2 == 0 else sb_E1)[:, c // 2, :]
            for c in range(4):
                nc.scalar.activation(E(c), sT(c), EXP, scale=0.125)

            # ---- [G; Z][d, pos] = sum_m [lk | 1][m, d] * E[m, pos] ----
            p_g = psA.tile([65, 64], F32, tag="g")
            for c in range(4):
                nc.tensor.matmul(
                    p_g[:], sb_lko[:, c, :], E(c),
                    start=(c == 0), stop=(c == 3),
                )
            sb_g = work.tile([65, 64], F32R, tag="gr")
            nc.vector.tensor_copy(sb_g[:], p_g[:])

            # ---- p_o[pos, 0:64] = (G.T @ Wvo)[pos, ch']; p_o[pos, 64] = Z
            p_o = psA.tile([64, 66], F32, tag="o")
            nc.tensor.matmul(p_o[:], sb_g[:], sb_wvo65,
                             start=True, stop=True)
            sb_rz = work.tile([64, 1], F32, tag="rz")
            nc.vector.reciprocal(sb_rz[:], p_o[:, 64:65])

            # scale + store in two halves: vector scales half 1 while the
            # scalar engine scales half 2 (Copy activation with a
            # per-partition scale), each feeding its own DMA queue.  Two
            # separate tiles — a shared tile would make Tile serialize
            # the cross-engine writers.
            sb_oa = work.tile([32, 64], F32, tag="oa")
            sb_ob = work.tile([64, 64], F32, tag="ob")
            nc.vector.tensor_scalar_mul(
                sb_oa[:, :], p_o[0:32, 0:64], sb_rz[0:32, :])
            nc.sync.dma_start(out_d.ap()[:32, :], sb_oa[:, :])
            nc.scalar.activation(
                sb_ob[32:64, :], p_o[32:64, 0:64],
                mybir.ActivationFunctionType.Copy, scale=sb_rz[32:64, :])
            nc.scalar.dma_start(out_d.ap()[32:, :], sb_ob[32:64, :])

    nc.compile()
    return nc


def _build_raw(wait_out: bool = True):
    """No-bias variant, raw bass (no TileContext): manual semaphores,
    no Tile entry/exit barriers or semaphore range-clear.  Same math and
    schedule as _build_fast, hand-synchronized.

    PSUM banks: z1 | z2 | wvo | sT0 | sT1 | g | o  (7 of 8).
    scoresT chunks alternate sT0/sT1 so exp(c) [ACT read] never touches
    the bank the PE is writing (c+1); the PE re-enters a bank only after
    waiting for the exp that read it.
    """
    from contextlib import ExitStack

    nc = bacc.Bacc("TRN2", target_bir_lowering=False, debug=False,
                   enable_asserts=False)

    xw_d = nc.dram_tensor("xw", [48, 288], F32R, kind="ExternalInput")
    w2k_d = nc.dram_tensor("w2k", [128, 4, 64], F32R, kind="ExternalInput")
    lkT_d = nc.dram_tensor("lkT", [64, 512], F32R, kind="ExternalInput")
    lko_d = nc.dram_tensor("lko", [128, 4, 65], F32R, kind="ExternalInput")
    tail_d = nc.dram_tensor("tail", [65, 256], F32R, kind="ExternalInput")
    out_d = nc.dram_tensor("out", [64, 64], F32, kind="ExternalOutput")

    es = ExitStack()
    sb_xw = es.enter_context(nc.sbuf_tensor([48, 288], F32R))
    sb_w2 = es.enter_context(nc.sbuf_tensor([128, 4, 64], F32R))
    sb_tail = es.enter_context(nc.sbuf_tensor([65, 256], F32R))
    sb_lkT = es.enter_context(nc.sbuf_tensor([64, 512], F32R))
    sb_lko = es.enter_context(nc.sbuf_tensor([128, 4, 65], F32R))
    sb_zero = es.enter_context(nc.sbuf_tensor([128, 18, 8], F32))
    imkw = es.enter_context(nc.sbuf_tensor([128, 18, 8], F32R))
    sb_zq = es.enter_context(nc.sbuf_tensor([64, 64], F32R))
    sb_E0 = es.enter_context(nc.sbuf_tensor([128, 2, 64], F32R))
    sb_E1 = es.enter_context(nc.sbuf_tensor([128, 2, 64], F32R))
    sb_g = es.enter_context(nc.sbuf_tensor([65, 64], F32R))
    sb_rz = es.enter_context(nc.sbuf_tensor([64, 1], F32))
    sb_oa = es.enter_context(nc.sbuf_tensor([32, 64], F32))
    sb_ob = es.enter_context(nc.sbuf_tensor([64, 64], F32))

    p_z1 = es.enter_context(nc.psum_tensor([32, 16, 16], F32))
    p_z2 = es.enter_context(nc.psum_tensor([64, 64], F32))
    p_wvo = es.enter_context(nc.psum_tensor([64, 64], F32))
    p_sT0 = es.enter_context(nc.psum_tensor([128, 2, 64], F32))
    p_sT1 = es.enter_context(nc.psum_tensor([128, 2, 64], F32))
    p_g = es.enter_context(nc.psum_tensor([65, 64], F32))
    p_o = es.enter_context(nc.psum_tensor([64, 66], F32))

    sXW = es.enter_context(nc.semaphore("sXW"))
    sTL = es.enter_context(nc.semaphore("sTL"))
    sLK = es.enter_context(nc.semaphore("sLK"))
    sW2 = es.enter_context(nc.semaphore("sW2"))
    sLO = es.enter_context(nc.semaphore("sLO"))
    sPE = es.enter_context(nc.semaphore("sPE"))
    sA = es.enter_context(nc.semaphore("sA"))
    sV = es.enter_context(nc.semaphore("sV"))
    sO1 = es.enter_context(nc.semaphore("sO1"))
    sO2 = es.enter_context(nc.semaphore("sO2"))

    sb_wvo65 = sb_tail[0:65, 128:194]
    sT = lambda c: (p_sT0 if c % 2 == 0 else p_sT1)[:, c // 2, :]
    E = lambda c: (sb_E0 if c % 2 == 0 else sb_E1)[:, c // 2, :]

    # ---- sync: three input DMAs, out half 1, final completion waits
    nc.sync.dma_start(sb_xw[:], xw_d.ap()).then_inc(sXW, 16)
    nc.sync.dma_start(sb_tail[:], tail_d.ap()).then_inc(sTL, 16)
    nc.sync.dma_start(sb_lkT[:], lkT_d.ap()).then_inc(sLK, 16)
    nc.sync.wait_ge(sV, 9)
    nc.sync.dma_start(out_d.ap()[:32, :], sb_oa[:]).then_inc(sO1, 16)
    if wait_out:
        nc.sync.wait_ge(sO1, 16)
        nc.sync.wait_ge(sO2, 16)

    # ---- gpsimd: lko DMA only
    nc.gpsimd.dma_start(sb_lko[:], lko_d.ap()).then_inc(sLO, 16)

    # ---- vector: zero prep, shifts, g cast, recip, out half 1
    nc.vector.memset(sb_zero[:], 0.0).then_inc(sV)                    # 1
    nc.vector.tensor_copy(imkw[:], sb_zero[:]).then_inc(sV)           # 2
    nc.vector.wait_ge(sPE, 1)
    nc.vector.tensor_scalar_max(
        imkw[0:32, 1:17, 1:8], p_z1[:, :, 1:15:2], 0.0).then_inc(sV)  # 3
    nc.vector.tensor_scalar_max(
        imkw[32:64, 1:17, 0:8], p_z1[:, :, 0:16:2], 0.0).then_inc(sV)
    nc.vector.tensor_scalar_max(
        imkw[64:96, 1:17, 0:8], p_z1[:, :, 1:16:2], 0.0).then_inc(sV)
    nc.vector.tensor_scalar_max(
        imkw[96:128, 1:17, 0:7], p_z1[:, :, 2:16:2], 0.0).then_inc(sV)  # 6
    nc.vector.wait_ge(sPE, 14)
    nc.vector.tensor_copy(sb_g[:], p_g[:]).then_inc(sV)               # 7
    nc.vector.wait_ge(sPE, 15)
    nc.vector.reciprocal(sb_rz[:], p_o[:, 64:65]).then_inc(sV)        # 8
    nc.vector.tensor_scalar_mul(
        sb_oa[:], p_o[0:32, 0:64], sb_rz[0:32, :]).then_inc(sV)       # 9

    # ---- scalar: w2k DMA, wvo copy, zq relu, 4 exps, out half 2
    nc.scalar.dma_start(sb_w2[:], w2k_d.ap()).then_inc(sW2, 16)
    nc.scalar.wait_ge(sPE, 2)
    nc.scalar.copy(sb_wvo65[0:64, 0:64], p_wvo[:]).then_inc(sA)       # 1
    nc.scalar.wait_ge(sPE, 6)
    nc.scalar.activation(sb_zq[:], p_z2[:], RELU).then_inc(sA)        # 2
    for c in range(4):
        nc.scalar.wait_ge(sPE, 7 + c)
        nc.scalar.activation(E(c), sT(c), EXP, scale=0.125).then_inc(sA)  # 3..6
    # mul2 serialized after mul1: V and A must not read PSUM bank 'o'
    # concurrently (same-bank V+A access is unarbitrated).
    nc.scalar.wait_ge(sV, 9)
    nc.scalar.activation(
        sb_ob[32:64, :], p_o[32:64, 0:64],
        mybir.ActivationFunctionType.Copy, scale=sb_rz[32:64, :]).then_inc(sA)
    # explicit wait: under relaxed ordering the engine dispatches the DMA
    # trigger before the preceding ACTIVATE's datapath drains, so without
    # a semaphore the HWDGE can read sb_ob mid-write.
    nc.scalar.wait_ge(sA, 7)
    nc.scalar.dma_start(out_d.ap()[32:, :], sb_ob[32:64, :]).then_inc(sO2, 16)

    # ---- tensor: conv1, wvo, conv2, scoresT, G, final
    nc.tensor.wait_ge(sXW, 16)
    nc.tensor.matmul(p_z1[:], sb_xw[:, 256:288], sb_xw[:, :256],
                     start=True, stop=True).then_inc(sPE)             # 1
    nc.tensor.wait_ge(sTL, 16)
    nc.tensor.matmul(p_wvo[:], sb_tail[0:64, 0:64], sb_tail[0:64, 64:128],
                     start=True, stop=True).then_inc(sPE)             # 2
    nc.tensor.wait_ge(sV, 6)
    nc.tensor.wait_ge(sW2, 16)
    for kh in range(4):
        nc.tensor.matmul(
            p_z2[:], sb_w2[:, kh, :], imkw[:, kh:min(kh + 16, 18):2, :],
            start=(kh == 0), stop=(kh == 3)).then_inc(sPE)            # 3..6
    nc.tensor.wait_ge(sA, 2)
    nc.tensor.wait_ge(sLK, 16)
    nc.tensor.matmul(sT(0), sb_lkT[:, 0:128], sb_zq[:],
                     start=True, stop=True).then_inc(sPE)             # 7
    nc.tensor.matmul(sT(1), sb_lkT[:, 128:256], sb_zq[:],
                     start=True, stop=True).then_inc(sPE)             # 8
    nc.tensor.wait_ge(sA, 3)  # exp(c0) released bank sT0
    nc.tensor.matmul(sT(2), sb_lkT[:, 256:384], sb_zq[:],
                     start=True, stop=True).then_inc(sPE)             # 9
    nc.tensor.wait_ge(sA, 4)  # exp(c1) released bank sT1
    nc.tensor.matmul(sT(3), sb_lkT[:, 384:512], sb_zq[:],
                     start=True, stop=True).then_inc(sPE)             # 10
    nc.tensor.wait_ge(sLO, 16)
    nc.tensor.matmul(p_g[:], sb_lko[:, 0, :], E(0),
                     start=True, stop=False).then_inc(sPE)            # 11
    nc.tensor.wait_ge(sA, 4)
    nc.tensor.matmul(p_g[:], sb_lko[:, 1, :], E(1),
                     start=False, stop=False).then_inc(sPE)           # 12
    nc.tensor.wait_ge(sA, 5)
    nc.tensor.matmul(p_g[:], sb_lko[:, 2, :], E(2),
                     start=False, stop=False).then_inc(sPE)           # 13
    nc.tensor.wait_ge(sA, 6)
    nc.tensor.matmul(p_g[:], sb_lko[:, 3, :], E(3),
                     start=False, stop=True).then_inc(sPE)            # 14
    nc.tensor.wait_ge(sV, 7)
    nc.tensor.matmul(p_o[:], sb_g[:], sb_wvo65,
                     start=True, stop=True).then_inc(sPE)             # 15

    es.close()
    nc.compile()
    return nc


def _build_bias():
    """General variant (nonzero biases): JVP via sign masks."""
    nc = bacc.Bacc("TRN2", target_bir_lowering=False, debug=False,
                   enable_asserts=False)

    x_im = nc.dram_tensor("x_im", [48, 256], F32R, kind="ExternalInput")
    w1r = nc.dram_tensor("w1r", [48, 32], F32R, kind="ExternalInput")
    w2k = nc.dram_tensor("w2k", [128, 4, 64], F32R, kind="ExternalInput")
    lkT = nc.dram_tensor("lkT", [64, 512], F32R, kind="ExternalInput")
    wvT = nc.dram_tensor("wvT", [64, 64], F32R, kind="ExternalInput")
    ident_d = nc.dram_tensor("ident", [64, 64], F32R, kind="ExternalInput")
    wo = nc.dram_tensor("wo", [64, 64], F32R, kind="ExternalInput")
    b1 = nc.dram_tensor("b1", [32, 1], F32, kind="ExternalInput")
    b2 = nc.dram_tensor("b2", [64, 1], F32, kind="ExternalInput")
    out_d = nc.dram_tensor("out", [64, 64], F32, kind="ExternalOutput")

    with tile.TileContext(nc) as tc:
        with (
            tc.tile_pool(name="consts", bufs=1) as consts,
            tc.tile_pool(name="work", bufs=1) as work,
            tc.tile_pool(name="psA", bufs=1, space="PSUM") as psA,
            tc.tile_pool(name="psT", bufs=2, space="PSUM") as psT,
        ):
            sb_xim = consts.tile([48, 256], F32R, tag="xim")
            nc.sync.dma_start(sb_xim[:24, :], x_im.ap()[:24, :])
            nc.scalar.dma_start(sb_xim[24:, :], x_im.ap()[24:, :])
            ident = consts.tile([64, 64], F32R, tag="ident")
            nc.gpsimd.dma_start(ident[:], ident_d.ap())
            sb_w1 = consts.tile([48, 32], F32R, tag="w1")
            nc.gpsimd.dma_start(sb_w1[:], w1r.ap())
            sb_w2 = consts.tile([128, 4, 64], F32R, tag="w2")
            nc.sync.dma_start(sb_w2[:, :2, :], w2k.ap()[:, :2, :])
            nc.scalar.dma_start(sb_w2[:, 2:, :], w2k.ap()[:, 2:, :])
            sb_lkT = consts.tile([64, 512], F32R, tag="lkT")
            nc.gpsimd.dma_start(sb_lkT[:, :256], lkT.ap()[:, :256])
            nc.sync.dma_start(sb_lkT[:, 256:], lkT.ap()[:, 256:])
            sb_wvT = consts.tile([64, 64], F32R, tag="wvT")
            nc.gpsimd.dma_start(sb_wvT[:], wvT.ap())
            sb_wo = consts.tile([64, 64], F32R, tag="wo")
            nc.scalar.dma_start(sb_wo[:], wo.ap())
            sb_b1 = consts.tile([32, 1], F32, tag="b1")
            nc.gpsimd.dma_start(sb_b1[:], b1.ap())
            sb_b2 = consts.tile([64, 1], F32, tag="b2")
            nc.gpsimd.dma_start(sb_b2[:], b2.ap())

            sb_zero = consts.tile([128, 18, 8], F32, tag="zero")
            nc.vector.memset(sb_zero[:], 0.0)
            sb_one = consts.tile([65, 2], F32R, tag="one")
            nc.vector.tensor_scalar_add(sb_one[64:65, :], sb_zero[64:65, 0, :2], 1.0)

            sb_lk = work.tile([128, 4, 65], F32R, tag="lk")
            nc.vector.tensor_scalar_add(sb_lk[:, :, 64:65],
                                        sb_zero[:, :4, :1], 1.0)

            # ---- conv1 ----
            p_z1 = psA.tile([32, 16, 16], F32, tag="a")
            nc.tensor.matmul(p_z1[:], sb_w1[:], sb_xim[:],
                             start=True, stop=True)

            def conv2(imkw, ps_tag):
                p = psA.tile([64, 64], F32, tag=ps_tag)
                for kh in range(4):
                    nc.tensor.matmul(
                        p[:],
                        sb_w2[:, kh, :],
                        imkw[:, kh:min(kh + 16, 18):2, :],
                        start=(kh == 0), stop=(kh == 3),
                    )
                return p

            # a1 = relu(z1 + b1); t1m = z1 * sign(a1)
            sb_a1 = work.tile([32, 16, 16], F32, tag="a1")
            nc.scalar.activation(
                sb_a1[:], p_z1[:], RELU, bias=sb_b1[:], scale=1.0)
            sb_m1 = work.tile([32, 16, 16], F32, tag="m1")
            nc.scalar.activation(
                sb_m1[:], sb_a1[:], mybir.ActivationFunctionType.Sign)
            sb_t1 = work.tile([32, 16, 16], F32, tag="t1")
            nc.vector.tensor_mul(sb_t1[:], p_z1[:], sb_m1[:])

            def shifts(dst, src):
                nc.vector.tensor_copy(dst[0:32, 1:17, 1:8], src[:, :, 1:15:2])
                nc.vector.tensor_copy(dst[32:64, 1:17, 0:8], src[:, :, 0:16:2])
                nc.vector.tensor_copy(dst[64:96, 1:17, 0:8], src[:, :, 1:16:2])
                nc.vector.tensor_copy(dst[96:128, 1:17, 0:7], src[:, :, 2:16:2])

            imkw = work.tile([128, 18, 8], F32R, tag="imkw")
            nc.vector.tensor_copy(imkw[:], sb_zero[:])
            shifts(imkw, sb_a1)
            p_z2 = conv2(imkw, "b")
            imkw2 = work.tile([128, 18, 8], F32R, tag="imkw2")
            nc.vector.tensor_copy(imkw2[:], sb_zero[:])
            shifts(imkw2, sb_t1)
            p_t2 = conv2(imkw2, "e")

            sb_zq = work.tile([64, 64], F32R, tag="zq")
            sb_z2r = work.tile([64, 64], F32, tag="z2r")
            nc.scalar.activation(
                sb_z2r[:], p_z2[:], RELU, bias=sb_b2[:], scale=1.0)
            sb_m2 = work.tile([64, 64], F32, tag="m2")
            nc.scalar.activation(
                sb_m2[:], sb_z2r[:], mybir.ActivationFunctionType.Sign)
            nc.vector.tensor_mul(sb_zq[:], p_t2[:], sb_m2[:])

            # ---- scoresT + lookup transposes ----
            p_sT = psA.tile([128, 4, 64], F32, tag="c")
            for c in range(4):
                nc.tensor.matmul(
                    p_sT[:, c, :],
                    sb_lkT[:, 128 * c:128 * (c + 1)], sb_zq[:],
                    start=True, stop=True,
                )
            for c in range(4):
                p_lk = psT.tile([128, 64], F32, tag="ptr")
                nc.tensor.matmul(
                    p_lk[:], sb_lkT[:, 128 * c:128 * (c + 1)], ident[:],
                    start=True, stop=True,
                )
                nc.scalar.copy(sb_lk[:, c, :64], p_lk[:])

            p_wvo = psA.tile([64, 64], F32, tag="d")
            nc.tensor.matmul(p_wvo[:], sb_wvT[:], sb_wo[:],
                             start=True, stop=True)
            sb_wvo = work.tile([64, 64], F32R, tag="wvo")
            nc.scalar.copy(sb_wvo[:], p_wvo[:])

            sb_E = work.tile([128, 4, 64], F32R, tag="E")
            nc.scalar.activation(sb_E[:], p_sT[:], EXP, scale=0.125)

            p_g = psA.tile([65, 64], F32, tag="d")
            for c in range(4):
                nc.tensor.matmul(
                    p_g[:], sb_lk[:, c, :], sb_E[:, c, :],
                    start=(c == 0), stop=(c == 3),
                )
            sb_g = work.tile([65, 64], F32R, tag="g")
            nc.vector.tensor_copy(sb_g[:], p_g[:])

            p_zT = psA.tile([64, 2], F32, tag="b")
            nc.tensor.matmul(p_zT[:], sb_g[64:65, :].bitcast(F32),
                             sb_one[64:65, :].bitcast(F32),
                             start=True, stop=True)
            sb_rz = work.tile([64, 1], F32, tag="rz")
            nc.vector.reciprocal(sb_rz[:], p_zT[:, :1])

            p_o = psA.tile([64, 64], F32, tag="a")
            nc.tensor.matmul(p_o[:], sb_g[:64, :], sb_wvo[:],
                             start=True, stop=True)
            sb_out = work.tile([64, 64], F32, tag="out")
            nc.vector.tensor_scalar_mul(sb_out[:], p_o[:], sb_rz[:])
            nc.sync.dma_start(out_d.ap()[:32, :], sb_out[:32, :])
            nc.scalar.dma_start(out_d.ap()[32:, :], sb_out[32:, :])

    nc.compile()
    return nc


def _get_nc(with_bias: bool):
    if with_bias not in _COMPILED:
        if with_bias:
            nc = _build_bias()
        elif os.environ.get("KERNEL_RAW"):
            nc = _build_raw(wait_out=not os.environ.get("KERNEL_NOWAIT"))
        else:
            nc = _build_fast()
        _COMPILED[with_bias] = nc
    return _COMPILED[with_bias]


def _im2col(x):
    """(B, 3, 32, 32) -> (B, 48, 256) im2col for conv1 (layout only)."""
    xp = np.zeros((B, CIN, 34, 34), np.float32)
    xp[:, :, 1:33, 1:33] = x
    xim = np.empty((B, CIN, 4, 4, 16, 16), np.float32)
    for kh in range(4):
        for kw in range(4):
            xim[:, :, kh, kw] = xp[:, :, kh:kh + 32:2, kw:kw + 32:2]
    return np.ascontiguousarray(xim.reshape(B, 48, 256))


def kernel(x, conv1_w, conv1_b, conv2_w, conv2_b, lookup, Wv, Wo):
    global last_exec_time_ns, last_trace_path
    x = np.asarray(x, np.float32)
    w1 = np.asarray(conv1_w, np.float32)
    b1 = np.asarray(conv1_b, np.float32)
    w2 = np.asarray(conv2_w, np.float32)
    b2 = np.asarray(conv2_b, np.float32)
    lk = np.ascontiguousarray(np.asarray(lookup, np.float32))
    wv = np.ascontiguousarray(np.asarray(Wv, np.float32))
    wo = np.ascontiguousarray(np.asarray(Wo, np.float32))

    with_bias = bool(np.any(b1 != 0.0) or np.any(b2 != 0.0))

    # host-side layout prep (no arithmetic): im2col of padded x, weight
    # transposes/re-chunking to the matmul-native layouts.
    xim = _im2col(x)
    w1r = np.ascontiguousarray(w1.transpose(1, 2, 3, 0).reshape(48, 32))
    # w2k[(kw*32+ci), kh, co] = w2[co, ci, kh, kw]
    w2k = np.ascontiguousarray(w2.transpose(3, 1, 2, 0).reshape(128, 4, 64))
    lkT = np.ascontiguousarray(lk.T)
    wvT = np.ascontiguousarray(wv.T)

    if not with_bias:
        # xw = [im2col | w1r]
        xw = np.concatenate([xim, np.broadcast_to(w1r, (B, 48, 32))], axis=2)
        xw = np.ascontiguousarray(xw)
        # lko[p, c, :64] = lookup[128c+p, :]; lko[p, c, 64] = 1
        lko = np.ones((128, 4, 65), np.float32)
        lko[:, :, :64] = lk.reshape(4, 128, C2).transpose(1, 0, 2)
        # tail = [wvT | wo | wvo65-border]: border is zeros with a lone 1
        # at [64, 64] of the 65x65 block (the Z-passthrough column).
        tail = np.zeros((65, 256), np.float32)
        tail[0:64, 0:64] = wvT
        tail[0:64, 64:128] = wo
        tail[64, 192] = 1.0
        shared = {"w2k": w2k, "lkT": lkT, "lko": lko, "tail": tail}
        in_maps = [dict(shared, xw=xw[c % B]) for c in range(N_CORES)]
    else:
        shared = {"w1r": w1r, "w2k": w2k, "lkT": lkT, "wvT": wvT, "wo": wo,
                  "ident": np.eye(64, dtype=np.float32),
                  "b1": np.ascontiguousarray(b1.reshape(32, 1)),
                  "b2": np.ascontiguousarray(b2.reshape(64, 1))}
        in_maps = [dict(shared, x_im=xim[c % B]) for c in range(N_CORES)]

    nc = _get_nc(with_bias)
    trace = bool(os.environ.get("KERNEL_TRACE"))
    res = run_bass_kernel_spmd(
        nc, in_maps, core_ids=list(range(N_CORES)),
        trace=trace, trace_cores=[0] if trace else None,
    )
    last_exec_time_ns = res.exec_time_ns
    if res.instructions_and_trace:
        last_trace_path = res.instructions_and_trace[1]

    # device emits (pos, ch') per sample; host transposes (layout only)
    out = np.stack([res.results[b]["out"].T for b in range(B)])
    return np.ascontiguousarray(out.reshape(B, C2, 8, 8))


# revision 17
# speedup vs baseline: 1.0839x; 1.0839x over previous
"""Trainium2 Bass kernel for nn_Block1_54279796687228 (retrieval_knn).

Math: the reference builds the full per-sample Jacobian J of the conv
encoder and contracts it with x.  For a conv+ReLU (piecewise-linear)
encoder, einsum(x, J) is exactly the JVP of the encoder at x in
direction x:

    z_q = m2 * conv2_nobias(m1 * conv1_nobias(x)),
    m1 = [conv1(x)+b1 > 0],  m2 = [conv2(relu(conv1(x)+b1))+b2 > 0]

With the zero biases produced by setup_inputs() this collapses to the
plain forward pass relu(conv2(relu(conv1(x)))).  Both variants are
implemented; the host picks based on the actual bias values.

Lowering (no-bias fast path):
  conv1 -> one K=48 matmul over a host-built im2col (layout only);
           im2col and w1r ride in ONE packed DMA.
  conv2 -> fold (ci,kw) into K=128: the ReLU+shift+f32r cast fuse into
           4 windowed vector ops straight out of PSUM; then 4
           accumulating matmuls (one per kh).
  Hopfield -> scores are computed directly TRANSPOSED, (mem, pos), as
           4 matmuls with lkT chunks stationary -- no softmax-axis
           transpose is ever needed.  The exp is split four ways over
           two alternating PSUM banks so each G matmul chases its own
           exp chunk while the PE streams.  The lookup chunks arrive
           in natural layout with an appended ones-column (layout only),
           so the 4 accumulating G matmuls produce [G; Z] in one go
           (Z = softmax denominator) and no on-device transposes of the
           lookup are needed.  The final projection matmul uses a
           [Wvo | e_Z | 0] operand (border pre-staged from the host,
           Wv@Wo folded on device off the critical path): its output
           column 64 IS Z^T, so no separate Z-transpose matmul exists
           (the zero pad column keeps the fp32r even-extent rule).
           out2 = (G.T @ [Wvo|e])[:, :64] / Z, emitted (pos, ch'); the
           host transposes each (64,64) sample for free.  The output is
           scaled and stored in two staggered halves (vector + scalar
           engines, separate tiles) feeding two DMA queues.

All matmuls run in float32r (single pass); ~3e-4 relative error
end-to-end vs the fp32 reference.

Sharding: pure data parallel over batch. Sample b runs on cores b and
b+4 (duplicates); host gathers from cores 0-3.
"""

import os
import numpy as np

# -- NTFF profile hook shim -------------------------------------------------
# bass_utils' trace path needs antenv.axon_hooks, which this image's antenv
# lacks. Register the ctypes-based hook from trn_agent_boot if available so
# trace=True / BASS_TRACE=1 works; degrade silently otherwise.
def _ensure_ntff_hook():
    try:
        import antenv.axon_hooks  # noqa: F401
        return
    except ImportError:
        pass
    try:
        import sys, types
        import antenv
        from trn_agent_boot.trn_boot import _ntff_profile_via_ctypes

        mod = types.ModuleType("antenv.axon_hooks")
        _h = [None]
        mod.set_axon_ntff_profile_hook = lambda h: _h.__setitem__(0, h)
        mod.get_axon_ntff_profile_hook = lambda: _h[0]
        sys.modules["antenv.axon_hooks"] = mod
        antenv.axon_hooks = mod
        so = "/opt/axon/libaxon_pjrt.so"
        if os.path.exists(so):
            mod.set_axon_ntff_profile_hook(_ntff_profile_via_ctypes(so))
    except Exception:
        pass


_ensure_ntff_hook()

import concourse.bacc as bacc
import concourse.bass as bass
import concourse.tile as tile
import concourse.mybir as mybir
from concourse.bass_utils import run_bass_kernel_spmd

F32 = mybir.dt.float32
F32R = mybir.dt.float32r
RELU = mybir.ActivationFunctionType.Relu
EXP = mybir.ActivationFunctionType.Exp

B, CIN, C1, C2, Q = 4, 3, 32, 64, 512  # batch, in-ch, conv1-ch, conv2-ch, memories
N_CORES = 8

_COMPILED = {}  # variant -> nc
last_exec_time_ns = None
last_trace_path = None


def _build_fast():
    """No-bias variant: forward pass + Hopfield, latency-optimized."""
    nc = bacc.Bacc("TRN2", target_bir_lowering=False, debug=False,
                   enable_asserts=False)

    # xw: im2col of padded x (48x256) with w1r (48x32) appended -> one DMA.
    xw_d = nc.dram_tensor("xw", [48, 288], F32R, kind="ExternalInput")
    w2k_d = nc.dram_tensor("w2k", [128, 4, 64], F32R, kind="ExternalInput")
    lkT_d = nc.dram_tensor("lkT", [64, 512], F32R, kind="ExternalInput")
    # lko: lookup in natural layout, chunked (128, 4, 65) with a ones col.
    lko_d = nc.dram_tensor("lko", [128, 4, 65], F32R, kind="ExternalInput")
    # tail: [wvT | wo | wvo66-border template | pad] (65 x 256; 1KB rows).
    tail_d = nc.dram_tensor("tail", [65, 256], F32R, kind="ExternalInput")
    out_d = nc.dram_tensor("out", [64, 64], F32, kind="ExternalOutput")

    with tile.TileContext(nc) as tc:
        with (
            tc.tile_pool(name="consts", bufs=1) as consts,
            tc.tile_pool(name="work", bufs=1) as work,
            tc.tile_pool(name="psA", bufs=1, space="PSUM") as psA,
        ):
            # ---- input DMAs, one per tensor, ordered by when they gate
            # compute.  HWDGE (sync/scalar) queues land ~1.6us after
            # desc-gen vs ~2.4us for SWDGE (gpsimd), so everything that
            # can stall the PE FIFO goes on sync: xw (conv1) first, then
            # tail (the wvo matmul sits early in the PE stream), then
            # lkT.  w2k (conv2) on scalar; only lko rides SWDGE.
            sb_xw = consts.tile([48, 288], F32R, tag="xw")
            nc.sync.dma_start(sb_xw[:], xw_d.ap())
            sb_w2 = consts.tile([128, 4, 64], F32R, tag="w2")
            nc.scalar.dma_start(sb_w2[:], w2k_d.ap())
            sb_tail = consts.tile([65, 256], F32R, tag="tail")
            nc.sync.dma_start(sb_tail[:], tail_d.ap())
            sb_lkT = consts.tile([64, 512], F32R, tag="lkT")
            nc.sync.dma_start(sb_lkT[:], lkT_d.ap())
            sb_lko = consts.tile([128, 4, 65], F32R, tag="lko")
            nc.gpsimd.dma_start(sb_lko[:], lko_d.ap())

            # f32r tiles cannot be memset directly; zero imkw's pad region
            # via a cast-copy from an f32 zero tile (early, no deps).
            sb_zero = consts.tile([128, 18, 8], F32, tag="zero")
            nc.vector.memset(sb_zero[:], 0.0)
            imkw = work.tile([128, 18, 8], F32R, tag="imkw")
            nc.vector.tensor_copy(imkw[:], sb_zero[:])

            # ---- conv1: (48,32).T @ (48,256) -> (32, 16, 16) ----
            p_z1 = psA.tile([32, 16, 16], F32, tag="z1")
            nc.tensor.matmul(p_z1[:], sb_xw[:, 256:288], sb_xw[:, :256],
                             start=True, stop=True)

            # ---- conv2 input: imkw[(kw,ci), row, c] = a1pad[ci, row, 2c+kw]
            # where a1pad = zero-pad(relu(z1)).  ReLU + shift + f32r cast
            # fuse into one windowed op per kw, straight from PSUM.  All
            # four stay on the vector engine: Tile serializes same-tile
            # writers across engines anyway, and a scalar-engine shift
            # would let unrelated scalar work interleave into the chain.
            nc.vector.tensor_scalar_max(
                imkw[0:32, 1:17, 1:8], p_z1[:, :, 1:15:2], 0.0)
            nc.vector.tensor_scalar_max(
                imkw[32:64, 1:17, 0:8], p_z1[:, :, 0:16:2], 0.0)
            nc.vector.tensor_scalar_max(
                imkw[64:96, 1:17, 0:8], p_z1[:, :, 1:16:2], 0.0)
            nc.vector.tensor_scalar_max(
                imkw[96:128, 1:17, 0:7], p_z1[:, :, 2:16:2], 0.0)

            # ---- conv2: 4 accumulating matmuls (one per kh) ----
            p_z2 = psA.tile([64, 64], F32, tag="z2")
            for kh in range(4):
                nc.tensor.matmul(
                    p_z2[:],
                    sb_w2[:, kh, :],
                    imkw[:, kh:min(kh + 16, 18):2, :],
                    start=(kh == 0), stop=(kh == 3),
                )

            # ---- Wvo = Wv @ Wo; PE is otherwise idle while zq is built.
            p_wvo = psA.tile([64, 64], F32, tag="wvo")
            nc.tensor.matmul(p_wvo[:], sb_tail[0:64, 0:64],
                             sb_tail[0:64, 64:128], start=True, stop=True)

            # zq relu on the scalar engine: shorter from PSUM and keeps
            # the vector queue free for the imkw shifts.
            sb_zq = work.tile([64, 64], F32R, tag="zq")
            nc.scalar.activation(sb_zq[:], p_z2[:], RELU)

            # wvo65 = [Wvo | e_Z]: border (zeros + lone 1 at [64,64]) came
            # from the host inside tail; the copy fills the Wvo block.
            sb_wvo65 = sb_tail[0:65, 128:194]
            nc.scalar.copy(sb_wvo65[0:64, 0:64], p_wvo[:])

            # ---- scoresT: (mem128, pos) chunks; lkT chunk stationary.
            # Chunks alternate between two PSUM tiles (= two banks) so
            # exp of chunk c can run while the PE writes chunk c+1: the
            # exp is split four ways and each G matmul chases its own
            # exp chunk.
            p_sT0 = psA.tile([128, 2, 64], F32, tag="sT0")
            p_sT1 = psA.tile([128, 2, 64], F32, tag="sT1")
            sT = lambda c: (p_sT0 if c % 2 == 0 else p_sT1)[:, c // 2, :]
            for c in range(4):
                nc.tensor.matmul(
                    sT(c),
                    sb_lkT[:, 128 * c:128 * (c + 1)], sb_zq[:],
                    start=True, stop=True,
                )

            # unnormalized softmax: E = exp(s/8).  |s/8| << 1 here, so
            # max-subtraction is unnecessary in fp32.
            sb_E0 = work.tile([128, 2, 64], F32R, tag="E0")
            sb_E1 = work.tile([128, 2, 64], F32R, tag="E1")
            E = lambda c: (sb_E0 if c % 2 == 0 else sb_E1)[:, c // 2, :]
            for c in range(4):
                nc.scalar.activation(E(c), sT(c), EXP, scale=0.125)

            # ---- [G; Z][d, pos] = sum_m [lk | 1][m, d] * E[m, pos] ----
            p_g = psA.tile([65, 64], F32, tag="g")
            for c in range(4):
                nc.tensor.matmul(
                    p_g[:], sb_lko[:, c, :], E(c),
                    start=(c == 0), stop=(c == 3),
                )
            sb_g = work.tile([65, 64], F32R, tag="gr")
            nc.vector.tensor_copy(sb_g[:], p_g[:])

            # ---- p_o[pos, 0:64] = (G.T @ Wvo)[pos, ch']; p_o[pos, 64] = Z
            p_o = psA.tile([64, 66], F32, tag="o")
            nc.tensor.matmul(p_o[:], sb_g[:], sb_wvo65,
                             start=True, stop=True)
            sb_rz = work.tile([64, 1], F32, tag="rz")
            nc.vector.reciprocal(sb_rz[:], p_o[:, 64:65])

            # scale + store in two halves: vector scales half 1 while the
            # scalar engine scales half 2 (Copy activation with a
            # per-partition scale), each feeding its own DMA queue.  Two
            # separate tiles — a shared tile would make Tile serialize
            # the cross-engine writers.
            sb_oa = work.tile([32, 64], F32, tag="oa")
            sb_ob = work.tile([64, 64], F32, tag="ob")
            nc.vector.tensor_scalar_mul(
                sb_oa[:, :], p_o[0:32, 0:64], sb_rz[0:32, :])
            nc.sync.dma_start(out_d.ap()[:32, :], sb_oa[:, :])
            nc.scalar.activation(
                sb_ob[32:64, :], p_o[32:64, 0:64],
                mybir.ActivationFunctionType.Copy, scale=sb_rz[32:64, :])
            nc.scalar.dma_start(out_d.ap()[32:, :], sb_ob[32:64, :])

    nc.compile()
    return nc


def _build_raw(wait_out: bool = True):
    """No-bias variant, raw bass (no TileContext): manual semaphores,
    no Tile entry/exit barriers or semaphore range-clear.  Same math and
    schedule as _build_fast, hand-synchronized.

    PSUM banks: z1 | z2 | wvo | sT0 | sT1 | g | o  (7 of 8).
    scoresT chunks alternate sT0/sT1 so exp(c) [ACT read] never touches
    the bank the PE is writing (c+1); the PE re-enters a bank only after
    waiting for the exp that read it.
    """
    from contextlib import ExitStack

    nc = bacc.Bacc("TRN2", target_bir_lowering=False, debug=False,
                   enable_asserts=False)

    xw_d = nc.dram_tensor("xw", [48, 288], F32R, kind="ExternalInput")
    w2k_d = nc.dram_tensor("w2k", [128, 4, 64], F32R, kind="ExternalInput")
    lkT_d = nc.dram_tensor("lkT", [64, 512], F32R, kind="ExternalInput")
    lko_d = nc.dram_tensor("lko", [128, 4, 65], F32R, kind="ExternalInput")
    tail_d = nc.dram_tensor("tail", [65, 256], F32R, kind="ExternalInput")
    out_d = nc.dram_tensor("out", [64, 64], F32, kind="ExternalOutput")

    es = ExitStack()
    sb_xw = es.enter_context(nc.sbuf_tensor([48, 288], F32R))
    sb_w2 = es.enter_context(nc.sbuf_tensor([128, 4, 64], F32R))
    sb_tail = es.enter_context(nc.sbuf_tensor([65, 256], F32R))
    sb_lkT = es.enter_context(nc.sbuf_tensor([64, 512], F32R))
    sb_lko = es.enter_context(nc.sbuf_tensor([128, 4, 65], F32R))
    sb_zero = es.enter_context(nc.sbuf_tensor([128, 18, 8], F32))
    imkw = es.enter_context(nc.sbuf_tensor([128, 18, 8], F32R))
    sb_zq = es.enter_context(nc.sbuf_tensor([64, 64], F32R))
    sb_E0 = es.enter_context(nc.sbuf_tensor([128, 2, 64], F32R))
    sb_E1 = es.enter_context(nc.sbuf_tensor([128, 2, 64], F32R))
    sb_g = es.enter_context(nc.sbuf_tensor([65, 64], F32R))
    sb_rz = es.enter_context(nc.sbuf_tensor([64, 1], F32))
    sb_oa = es.enter_context(nc.sbuf_tensor([32, 64], F32))
    sb_ob = es.enter_context(nc.sbuf_tensor([64, 64], F32))

    p_z1 = es.enter_context(nc.psum_tensor([32, 16, 16], F32))
    p_z2 = es.enter_context(nc.psum_tensor([64, 64], F32))
    p_wvo = es.enter_context(nc.psum_tensor([64, 64], F32))
    p_sT0 = es.enter_context(nc.psum_tensor([128, 2, 64], F32))
    p_sT1 = es.enter_context(nc.psum_tensor([128, 2, 64], F32))
    p_g = es.enter_context(nc.psum_tensor([65, 64], F32))
    p_o = es.enter_context(nc.psum_tensor([64, 66], F32))

    sXW = es.enter_context(nc.semaphore("sXW"))
    sTL = es.enter_context(nc.semaphore("sTL"))
    sLK = es.enter_context(nc.semaphore("sLK"))
    sW2 = es.enter_context(nc.semaphore("sW2"))
    sLO = es.enter_context(nc.semaphore("sLO"))
    sPE = es.enter_context(nc.semaphore("sPE"))
    sA = es.enter_context(nc.semaphore("sA"))
    sV = es.enter_context(nc.semaphore("sV"))
    sO1 = es.enter_context(nc.semaphore("sO1"))
    sO2 = es.enter_context(nc.semaphore("sO2"))

    sb_wvo65 = sb_tail[0:65, 128:194]
    sT = lambda c: (p_sT0 if c % 2 == 0 else p_sT1)[:, c // 2, :]
    E = lambda c: (sb_E0 if c % 2 == 0 else sb_E1)[:, c // 2, :]

    # ---- sync: three input DMAs, out half 1, final completion waits
    nc.sync.dma_start(sb_xw[:], xw_d.ap()).then_inc(sXW, 16)
    nc.sync.dma_start(sb_tail[:], tail_d.ap()).then_inc(sTL, 16)
    nc.sync.dma_start(sb_lkT[:], lkT_d.ap()).then_inc(sLK, 16)
    nc.sync.wait_ge(sV, 9)
    nc.sync.dma_start(out_d.ap()[:32, :], sb_oa[:]).then_inc(sO1, 16)
    if wait_out:
        nc.sync.wait_ge(sO1, 16)
        nc.sync.wait_ge(sO2, 16)

    # ---- gpsimd: lko DMA only
    nc.gpsimd.dma_start(sb_lko[:], lko_d.ap()).then_inc(sLO, 16)

    # ---- vector: zero prep, shifts, g cast, recip, out half 1
    # Under relaxed ordering mode, engine-queue ops may overlap/reorder:
    # every op chains on the previous one's completion sem (as Tile does)
    # in addition to its cross-engine waits.
    nc.vector.memset(sb_zero[:], 0.0).then_inc(sV)                    # 1
    nc.vector.wait_ge(sV, 1)
    nc.vector.tensor_copy(imkw[:], sb_zero[:]).then_inc(sV)           # 2
    nc.vector.wait_ge(sV, 2)
    nc.vector.wait_ge(sPE, 1)
    nc.vector.tensor_scalar_max(
        imkw[0:32, 1:17, 1:8], p_z1[:, :, 1:15:2], 0.0).then_inc(sV)  # 3
    nc.vector.wait_ge(sV, 3)
    nc.vector.tensor_scalar_max(
        imkw[32:64, 1:17, 0:8], p_z1[:, :, 0:16:2], 0.0).then_inc(sV)
    nc.vector.wait_ge(sV, 4)
    nc.vector.tensor_scalar_max(
        imkw[64:96, 1:17, 0:8], p_z1[:, :, 1:16:2], 0.0).then_inc(sV)
    nc.vector.wait_ge(sV, 5)
    nc.vector.tensor_scalar_max(
        imkw[96:128, 1:17, 0:7], p_z1[:, :, 2:16:2], 0.0).then_inc(sV)  # 6
    nc.vector.wait_ge(sV, 6)
    nc.vector.wait_ge(sPE, 14)
    nc.vector.tensor_copy(sb_g[:], p_g[:]).then_inc(sV)               # 7
    nc.vector.wait_ge(sV, 7)
    nc.vector.wait_ge(sPE, 15)
    nc.vector.reciprocal(sb_rz[:], p_o[:, 64:65]).then_inc(sV)        # 8
    nc.vector.wait_ge(sV, 8)
    nc.vector.tensor_scalar_mul(
        sb_oa[:], p_o[0:32, 0:64], sb_rz[0:32, :]).then_inc(sV)       # 9

    # ---- scalar: w2k DMA, wvo copy, zq relu, 4 exps, out half 2
    # Same-engine chain on sA throughout (see vector comment).
    nc.scalar.dma_start(sb_w2[:], w2k_d.ap()).then_inc(sW2, 16)
    nc.scalar.wait_ge(sPE, 2)
    nc.scalar.copy(sb_wvo65[0:64, 0:64], p_wvo[:]).then_inc(sA)       # 1
    nc.scalar.wait_ge(sA, 1)
    nc.scalar.wait_ge(sPE, 6)
    nc.scalar.activation(sb_zq[:], p_z2[:], RELU).then_inc(sA)        # 2
    for c in range(4):
        nc.scalar.wait_ge(sA, 2 + c)
        nc.scalar.wait_ge(sPE, 7 + c)
        nc.scalar.activation(E(c), sT(c), EXP, scale=0.125).then_inc(sA)  # 3..6
    # mul2 serialized after mul1: V and A must not read PSUM bank 'o'
    # concurrently (same-bank V+A access is unarbitrated).
    nc.scalar.wait_ge(sA, 6)
    nc.scalar.wait_ge(sV, 9)
    nc.scalar.activation(
        sb_ob[32:64, :], p_o[32:64, 0:64],
        mybir.ActivationFunctionType.Copy, scale=sb_rz[32:64, :]).then_inc(sA)
    # explicit wait: under relaxed ordering the engine dispatches the DMA
    # trigger before the preceding ACTIVATE's datapath drains, so without
    # a semaphore the HWDGE can read sb_ob mid-write.
    nc.scalar.wait_ge(sA, 7)
    nc.scalar.dma_start(out_d.ap()[32:, :], sb_ob[32:64, :]).then_inc(sO2, 16)

    # ---- tensor: conv1, wvo, conv2, scoresT, G, final
    nc.tensor.wait_ge(sXW, 16)
    nc.tensor.matmul(p_z1[:], sb_xw[:, 256:288], sb_xw[:, :256],
                     start=True, stop=True).then_inc(sPE)             # 1
    nc.tensor.wait_ge(sTL, 16)
    nc.tensor.matmul(p_wvo[:], sb_tail[0:64, 0:64], sb_tail[0:64, 64:128],
                     start=True, stop=True).then_inc(sPE)             # 2
    nc.tensor.wait_ge(sV, 6)
    nc.tensor.wait_ge(sW2, 16)
    for kh in range(4):
        nc.tensor.matmul(
            p_z2[:], sb_w2[:, kh, :], imkw[:, kh:min(kh + 16, 18):2, :],
            start=(kh == 0), stop=(kh == 3)).then_inc(sPE)            # 3..6
    nc.tensor.wait_ge(sA, 2)
    nc.tensor.wait_ge(sLK, 16)
    nc.tensor.matmul(sT(0), sb_lkT[:, 0:128], sb_zq[:],
                     start=True, stop=True).then_inc(sPE)             # 7
    nc.tensor.matmul(sT(1), sb_lkT[:, 128:256], sb_zq[:],
                     start=True, stop=True).then_inc(sPE)             # 8
    nc.tensor.wait_ge(sA, 3)  # exp(c0) released bank sT0
    nc.tensor.matmul(sT(2), sb_lkT[:, 256:384], sb_zq[:],
                     start=True, stop=True).then_inc(sPE)             # 9
    nc.tensor.wait_ge(sA, 4)  # exp(c1) released bank sT1
    nc.tensor.matmul(sT(3), sb_lkT[:, 384:512], sb_zq[:],
                     start=True, stop=True).then_inc(sPE)             # 10
    nc.tensor.wait_ge(sLO, 16)
    nc.tensor.matmul(p_g[:], sb_lko[:, 0, :], E(0),
                     start=True, stop=False).then_inc(sPE)            # 11
    nc.tensor.wait_ge(sA, 4)
    nc.tensor.matmul(p_g[:], sb_lko[:, 1, :], E(1),
                     start=False, stop=False).then_inc(sPE)           # 12
    nc.tensor.wait_ge(sA, 5)
    nc.tensor.matmul(p_g[:], sb_lko[:, 2, :], E(2),
                     start=False, stop=False).then_inc(sPE)           # 13
    nc.tensor.wait_ge(sA, 6)
    nc.tensor.matmul(p_g[:], sb_lko[:, 3, :], E(3),
                     start=False, stop=True).then_inc(sPE)            # 14
    nc.tensor.wait_ge(sV, 7)
    nc.tensor.matmul(p_o[:], sb_g[:], sb_wvo65,
                     start=True, stop=True).then_inc(sPE)             # 15

    es.close()
    nc.compile()
    return nc


def _build_bias():
    """General variant (nonzero biases): JVP via sign masks."""
    nc = bacc.Bacc("TRN2", target_bir_lowering=False, debug=False,
                   enable_asserts=False)

    x_im = nc.dram_tensor("x_im", [48, 256], F32R, kind="ExternalInput")
    w1r = nc.dram_tensor("w1r", [48, 32], F32R, kind="ExternalInput")
    w2k = nc.dram_tensor("w2k", [128, 4, 64], F32R, kind="ExternalInput")
    lkT = nc.dram_tensor("lkT", [64, 512], F32R, kind="ExternalInput")
    wvT = nc.dram_tensor("wvT", [64, 64], F32R, kind="ExternalInput")
    ident_d = nc.dram_tensor("ident", [64, 64], F32R, kind="ExternalInput")
    wo = nc.dram_tensor("wo", [64, 64], F32R, kind="ExternalInput")
    b1 = nc.dram_tensor("b1", [32, 1], F32, kind="ExternalInput")
    b2 = nc.dram_tensor("b2", [64, 1], F32, kind="ExternalInput")
    out_d = nc.dram_tensor("out", [64, 64], F32, kind="ExternalOutput")

    with tile.TileContext(nc) as tc:
        with (
            tc.tile_pool(name="consts", bufs=1) as consts,
            tc.tile_pool(name="work", bufs=1) as work,
            tc.tile_pool(name="psA", bufs=1, space="PSUM") as psA,
            tc.tile_pool(name="psT", bufs=2, space="PSUM") as psT,
        ):
            sb_xim = consts.tile([48, 256], F32R, tag="xim")
            nc.sync.dma_start(sb_xim[:24, :], x_im.ap()[:24, :])
            nc.scalar.dma_start(sb_xim[24:, :], x_im.ap()[24:, :])
            ident = consts.tile([64, 64], F32R, tag="ident")
            nc.gpsimd.dma_start(ident[:], ident_d.ap())
            sb_w1 = consts.tile([48, 32], F32R, tag="w1")
            nc.gpsimd.dma_start(sb_w1[:], w1r.ap())
            sb_w2 = consts.tile([128, 4, 64], F32R, tag="w2")
            nc.sync.dma_start(sb_w2[:, :2, :], w2k.ap()[:, :2, :])
            nc.scalar.dma_start(sb_w2[:, 2:, :], w2k.ap()[:, 2:, :])
            sb_lkT = consts.tile([64, 512], F32R, tag="lkT")
            nc.gpsimd.dma_start(sb_lkT[:, :256], lkT.ap()[:, :256])
            nc.sync.dma_start(sb_lkT[:, 256:], lkT.ap()[:, 256:])
            sb_wvT = consts.tile([64, 64], F32R, tag="wvT")
            nc.gpsimd.dma_start(sb_wvT[:], wvT.ap())
            sb_wo = consts.tile([64, 64], F32R, tag="wo")
            nc.scalar.dma_start(sb_wo[:], wo.ap())
            sb_b1 = consts.tile([32, 1], F32, tag="b1")
            nc.gpsimd.dma_start(sb_b1[:], b1.ap())
            sb_b2 = consts.tile([64, 1], F32, tag="b2")
            nc.gpsimd.dma_start(sb_b2[:], b2.ap())

            sb_zero = consts.tile([128, 18, 8], F32, tag="zero")
            nc.vector.memset(sb_zero[:], 0.0)
            sb_one = consts.tile([65, 2], F32R, tag="one")
            nc.vector.tensor_scalar_add(sb_one[64:65, :], sb_zero[64:65, 0, :2], 1.0)

            sb_lk = work.tile([128, 4, 65], F32R, tag="lk")
            nc.vector.tensor_scalar_add(sb_lk[:, :, 64:65],
                                        sb_zero[:, :4, :1], 1.0)

            # ---- conv1 ----
            p_z1 = psA.tile([32, 16, 16], F32, tag="a")
            nc.tensor.matmul(p_z1[:], sb_w1[:], sb_xim[:],
                             start=True, stop=True)

            def conv2(imkw, ps_tag):
                p = psA.tile([64, 64], F32, tag=ps_tag)
                for kh in range(4):
                    nc.tensor.matmul(
                        p[:],
                        sb_w2[:, kh, :],
                        imkw[:, kh:min(kh + 16, 18):2, :],
                        start=(kh == 0), stop=(kh == 3),
                    )
                return p

            # a1 = relu(z1 + b1); t1m = z1 * sign(a1)
            sb_a1 = work.tile([32, 16, 16], F32, tag="a1")
            nc.scalar.activation(
                sb_a1[:], p_z1[:], RELU, bias=sb_b1[:], scale=1.0)
            sb_m1 = work.tile([32, 16, 16], F32, tag="m1")
            nc.scalar.activation(
                sb_m1[:], sb_a1[:], mybir.ActivationFunctionType.Sign)
            sb_t1 = work.tile([32, 16, 16], F32, tag="t1")
            nc.vector.tensor_mul(sb_t1[:], p_z1[:], sb_m1[:])

            def shifts(dst, src):
                nc.vector.tensor_copy(dst[0:32, 1:17, 1:8], src[:, :, 1:15:2])
                nc.vector.tensor_copy(dst[32:64, 1:17, 0:8], src[:, :, 0:16:2])
                nc.vector.tensor_copy(dst[64:96, 1:17, 0:8], src[:, :, 1:16:2])
                nc.vector.tensor_copy(dst[96:128, 1:17, 0:7], src[:, :, 2:16:2])

            imkw = work.tile([128, 18, 8], F32R, tag="imkw")
            nc.vector.tensor_copy(imkw[:], sb_zero[:])
            shifts(imkw, sb_a1)
            p_z2 = conv2(imkw, "b")
            imkw2 = work.tile([128, 18, 8], F32R, tag="imkw2")
            nc.vector.tensor_copy(imkw2[:], sb_zero[:])
            shifts(imkw2, sb_t1)
            p_t2 = conv2(imkw2, "e")

            sb_zq = work.tile([64, 64], F32R, tag="zq")
            sb_z2r = work.tile([64, 64], F32, tag="z2r")
            nc.scalar.activation(
                sb_z2r[:], p_z2[:], RELU, bias=sb_b2[:], scale=1.0)
            sb_m2 = work.tile([64, 64], F32, tag="m2")
            nc.scalar.activation(
                sb_m2[:], sb_z2r[:], mybir.ActivationFunctionType.Sign)
            nc.vector.tensor_mul(sb_zq[:], p_t2[:], sb_m2[:])

            # ---- scoresT + lookup transposes ----
            p_sT = psA.tile([128, 4, 64], F32, tag="c")
            for c in range(4):
                nc.tensor.matmul(
                    p_sT[:, c, :],
                    sb_lkT[:, 128 * c:128 * (c + 1)], sb_zq[:],
                    start=True, stop=True,
                )
            for c in range(4):
                p_lk = psT.tile([128, 64], F32, tag="ptr")
                nc.tensor.matmul(
                    p_lk[:], sb_lkT[:, 128 * c:128 * (c + 1)], ident[:],
                    start=True, stop=True,
                )
                nc.scalar.copy(sb_lk[:, c, :64], p_lk[:])

            p_wvo = psA.tile([64, 64], F32, tag="d")
            nc.tensor.matmul(p_wvo[:], sb_wvT[:], sb_wo[:],
                             start=True, stop=True)
            sb_wvo = work.tile([64, 64], F32R, tag="wvo")
            nc.scalar.copy(sb_wvo[:], p_wvo[:])

            sb_E = work.tile([128, 4, 64], F32R, tag="E")
            nc.scalar.activation(sb_E[:], p_sT[:], EXP, scale=0.125)

            p_g = psA.tile([65, 64], F32, tag="d")
            for c in range(4):
                nc.tensor.matmul(
                    p_g[:], sb_lk[:, c, :], sb_E[:, c, :],
                    start=(c == 0), stop=(c == 3),
                )
            sb_g = work.tile([65, 64], F32R, tag="g")
            nc.vector.tensor_copy(sb_g[:], p_g[:])

            p_zT = psA.tile([64, 2], F32, tag="b")
            nc.tensor.matmul(p_zT[:], sb_g[64:65, :].bitcast(F32),
                             sb_one[64:65, :].bitcast(F32),
                             start=True, stop=True)
            sb_rz = work.tile([64, 1], F32, tag="rz")
            nc.vector.reciprocal(sb_rz[:], p_zT[:, :1])

            p_o = psA.tile([64, 64], F32, tag="a")
            nc.tensor.matmul(p_o[:], sb_g[:64, :], sb_wvo[:],
                             start=True, stop=True)
            sb_out = work.tile([64, 64], F32, tag="out")
            nc.vector.tensor_scalar_mul(sb_out[:], p_o[:], sb_rz[:])
            nc.sync.dma_start(out_d.ap()[:32, :], sb_out[:32, :])
            nc.scalar.dma_start(out_d.ap()[32:, :], sb_out[32:, :])

    nc.compile()
    return nc


def _get_nc(with_bias: bool):
    if with_bias not in _COMPILED:
        if with_bias:
            nc = _build_bias()
        elif os.environ.get("KERNEL_RAW"):
            nc = _build_raw(wait_out=not os.environ.get("KERNEL_NOWAIT"))
        else:
            nc = _build_fast()
        _COMPILED[with_bias] = nc
    return _COMPILED[with_bias]


def _im2col(x):
    """(B, 3, 32, 32) -> (B, 48, 256) im2col for conv1 (layout only)."""
    xp = np.zeros((B, CIN, 34, 34), np.float32)
    xp[:, :, 1:33, 1:33] = x
    xim = np.empty((B, CIN, 4, 4, 16, 16), np.float32)
    for kh in range(4):
        for kw in range(4):
            xim[:, :, kh, kw] = xp[:, :, kh:kh + 32:2, kw:kw + 32:2]
    return np.ascontiguousarray(xim.reshape(B, 48, 256))


def kernel(x, conv1_w, conv1_b, conv2_w, conv2_b, lookup, Wv, Wo):
    global last_exec_time_ns, last_trace_path
    x = np.asarray(x, np.float32)
    w1 = np.asarray(conv1_w, np.float32)
    b1 = np.asarray(conv1_b, np.float32)
    w2 = np.asarray(conv2_w, np.float32)
    b2 = np.asarray(conv2_b, np.float32)
    lk = np.ascontiguousarray(np.asarray(lookup, np.float32))
    wv = np.ascontiguousarray(np.asarray(Wv, np.float32))
    wo = np.ascontiguousarray(np.asarray(Wo, np.float32))

    with_bias = bool(np.any(b1 != 0.0) or np.any(b2 != 0.0))

    # host-side layout prep (no arithmetic): im2col of padded x, weight
    # transposes/re-chunking to the matmul-native layouts.
    xim = _im2col(x)
    w1r = np.ascontiguousarray(w1.transpose(1, 2, 3, 0).reshape(48, 32))
    # w2k[(kw*32+ci), kh, co] = w2[co, ci, kh, kw]
    w2k = np.ascontiguousarray(w2.transpose(3, 1, 2, 0).reshape(128, 4, 64))
    lkT = np.ascontiguousarray(lk.T)
    wvT = np.ascontiguousarray(wv.T)

    if not with_bias:
        # xw = [im2col | w1r]
        xw = np.concatenate([xim, np.broadcast_to(w1r, (B, 48, 32))], axis=2)
        xw = np.ascontiguousarray(xw)
        # lko[p, c, :64] = lookup[128c+p, :]; lko[p, c, 64] = 1
        lko = np.ones((128, 4, 65), np.float32)
        lko[:, :, :64] = lk.reshape(4, 128, C2).transpose(1, 0, 2)
        # tail = [wvT | wo | wvo65-border]: border is zeros with a lone 1
        # at [64, 64] of the 65x65 block (the Z-passthrough column).
        tail = np.zeros((65, 256), np.float32)
        tail[0:64, 0:64] = wvT
        tail[0:64, 64:128] = wo
        tail[64, 192] = 1.0
        shared = {"w2k": w2k, "lkT": lkT, "lko": lko, "tail": tail}
        in_maps = [dict(shared, xw=xw[c % B]) for c in range(N_CORES)]
    else:
        shared = {"w1r": w1r, "w2k": w2k, "lkT": lkT, "wvT": wvT, "wo": wo,
                  "ident": np.eye(64, dtype=np.float32),
                  "b1": np.ascontiguousarray(b1.reshape(32, 1)),
                  "b2": np.ascontiguousarray(b2.reshape(64, 1))}
        in_maps = [dict(shared, x_im=xim[c % B]) for c in range(N_CORES)]

    nc = _get_nc(with_bias)
    trace = bool(os.environ.get("KERNEL_TRACE"))
    res = run_bass_kernel_spmd(
        nc, in_maps, core_ids=list(range(N_CORES)),
        trace=trace, trace_cores=[0] if trace else None,
    )
    last_exec_time_ns = res.exec_time_ns
    if res.instructions_and_trace:
        last_trace_path = res.instructions_and_trace[1]

    # device emits (pos, ch') per sample; host transposes (layout only)
    out = np.stack([res.results[b]["out"].T for b in range(B)])
    return np.ascontiguousarray(out.reshape(B, C2, 8, 8))


# revision 19
# speedup vs baseline: 1.0852x; 1.0012x over previous
"""Trainium2 Bass kernel for nn_Block1_54279796687228 (retrieval_knn).

Math: the reference builds the full per-sample Jacobian J of the conv
encoder and contracts it with x.  For a conv+ReLU (piecewise-linear)
encoder, einsum(x, J) is exactly the JVP of the encoder at x in
direction x:

    z_q = m2 * conv2_nobias(m1 * conv1_nobias(x)),
    m1 = [conv1(x)+b1 > 0],  m2 = [conv2(relu(conv1(x)+b1))+b2 > 0]

With the zero biases produced by setup_inputs() this collapses to the
plain forward pass relu(conv2(relu(conv1(x)))).  Both variants are
implemented; the host picks based on the actual bias values.

Lowering (no-bias fast path):
  conv1 -> one K=48 matmul over a host-built im2col (layout only);
           im2col and w1r ride in ONE packed DMA.
  conv2 -> fold (ci,kw) into K=128: the ReLU+shift+f32r cast fuse into
           4 windowed vector ops straight out of PSUM; then 4
           accumulating matmuls (one per kh).
  Hopfield -> scores are computed directly TRANSPOSED, (mem, pos), as
           4 matmuls with lkT chunks stationary -- no softmax-axis
           transpose is ever needed.  The exp is split four ways over
           two alternating PSUM banks so each G matmul chases its own
           exp chunk while the PE streams.  The lookup chunks arrive
           in natural layout with an appended ones-column (layout only),
           so the 4 accumulating G matmuls produce [G; Z] in one go
           (Z = softmax denominator) and no on-device transposes of the
           lookup are needed.  The final projection matmul uses a
           [Wvo | e_Z | 0] operand (border pre-staged from the host,
           Wv@Wo folded on device off the critical path): its output
           column 64 IS Z^T, so no separate Z-transpose matmul exists
           (the zero pad column keeps the fp32r even-extent rule).
           out2 = (G.T @ [Wvo|e])[:, :64] / Z, emitted (pos, ch'); the
           host transposes each (64,64) sample for free.  The output is
           scaled and stored in two staggered halves (vector + scalar
           engines, separate tiles) feeding two DMA queues.

All matmuls run in float32r (single pass); ~3e-4 relative error
end-to-end vs the fp32 reference.

Sharding: pure data parallel over batch. Sample b runs on cores b and
b+4 (duplicates); host gathers from cores 0-3.
"""

import os
import numpy as np

# -- NTFF profile hook shim -------------------------------------------------
# bass_utils' trace path needs antenv.axon_hooks, which this image's antenv
# lacks. Register the ctypes-based hook from trn_agent_boot if available so
# trace=True / BASS_TRACE=1 works; degrade silently otherwise.
def _ensure_ntff_hook():
    try:
        import antenv.axon_hooks  # noqa: F401
        return
    except ImportError:
        pass
    try:
        import sys, types
        import antenv
        from trn_agent_boot.trn_boot import _ntff_profile_via_ctypes

        mod = types.ModuleType("antenv.axon_hooks")
        _h = [None]
        mod.set_axon_ntff_profile_hook = lambda h: _h.__setitem__(0, h)
        mod.get_axon_ntff_profile_hook = lambda: _h[0]
        sys.modules["antenv.axon_hooks"] = mod
        antenv.axon_hooks = mod
        so = "/opt/axon/libaxon_pjrt.so"
        if os.path.exists(so):
            mod.set_axon_ntff_profile_hook(_ntff_profile_via_ctypes(so))
    except Exception:
        pass


_ensure_ntff_hook()

import concourse.bacc as bacc
import concourse.bass as bass
import concourse.tile as tile
import concourse.mybir as mybir
from concourse.bass_utils import run_bass_kernel_spmd

F32 = mybir.dt.float32
F32R = mybir.dt.float32r
RELU = mybir.ActivationFunctionType.Relu
EXP = mybir.ActivationFunctionType.Exp

B, CIN, C1, C2, Q = 4, 3, 32, 64, 512  # batch, in-ch, conv1-ch, conv2-ch, memories
N_CORES = 8

_COMPILED = {}  # variant -> nc
last_exec_time_ns = None
last_trace_path = None


def _build_fast():
    """No-bias variant: forward pass + Hopfield, latency-optimized."""
    nc = bacc.Bacc("TRN2", target_bir_lowering=False, debug=False,
                   enable_asserts=False)

    # xw: im2col of padded x (48x256) with w1r (48x32) appended -> one DMA.
    xw_d = nc.dram_tensor("xw", [48, 288], F32R, kind="ExternalInput")
    w2k_d = nc.dram_tensor("w2k", [128, 4, 64], F32R, kind="ExternalInput")
    lkT_d = nc.dram_tensor("lkT", [64, 512], F32R, kind="ExternalInput")
    # lko: lookup in natural layout, chunked (128, 4, 65) with a ones col.
    lko_d = nc.dram_tensor("lko", [128, 4, 65], F32R, kind="ExternalInput")
    # tail: [wvT | wo | wvo66-border template | pad] (65 x 256; 1KB rows).
    tail_d = nc.dram_tensor("tail", [65, 256], F32R, kind="ExternalInput")
    out_d = nc.dram_tensor("out", [64, 64], F32, kind="ExternalOutput")

    with tile.TileContext(nc) as tc:
        with (
            tc.tile_pool(name="consts", bufs=1) as consts,
            tc.tile_pool(name="work", bufs=1) as work,
            tc.tile_pool(name="psA", bufs=1, space="PSUM") as psA,
        ):
            # ---- input DMAs, one per tensor, ordered by when they gate
            # compute.  HWDGE (sync/scalar) queues land ~1.6us after
            # desc-gen vs ~2.4us for SWDGE (gpsimd), so everything that
            # can stall the PE FIFO goes on sync: xw (conv1) first, then
            # tail (the wvo matmul sits early in the PE stream), then
            # lkT.  w2k (conv2) on scalar; only lko rides SWDGE.
            sb_xw = consts.tile([48, 288], F32R, tag="xw")
            nc.sync.dma_start(sb_xw[:], xw_d.ap())
            sb_w2 = consts.tile([128, 4, 64], F32R, tag="w2")
            nc.scalar.dma_start(sb_w2[:], w2k_d.ap())
            sb_tail = consts.tile([65, 256], F32R, tag="tail")
            nc.sync.dma_start(sb_tail[:], tail_d.ap())
            sb_lkT = consts.tile([64, 512], F32R, tag="lkT")
            nc.sync.dma_start(sb_lkT[:], lkT_d.ap())
            sb_lko = consts.tile([128, 4, 65], F32R, tag="lko")
            nc.gpsimd.dma_start(sb_lko[:], lko_d.ap())

            # f32r tiles cannot be memset directly; zero imkw's pad region
            # via a cast-copy from an f32 zero tile (early, no deps).
            sb_zero = consts.tile([128, 18, 8], F32, tag="zero")
            nc.vector.memset(sb_zero[:], 0.0)
            imkw = work.tile([128, 18, 8], F32R, tag="imkw")
            nc.vector.tensor_copy(imkw[:], sb_zero[:])

            # ---- conv1: (48,32).T @ (48,256) -> (32, 16, 16) ----
            p_z1 = psA.tile([32, 16, 16], F32, tag="z1")
            nc.tensor.matmul(p_z1[:], sb_xw[:, 256:288], sb_xw[:, :256],
                             start=True, stop=True)

            # ---- conv2 input: imkw[(kw,ci), row, c] = a1pad[ci, row, 2c+kw]
            # where a1pad = zero-pad(relu(z1)).  ReLU + shift + f32r cast
            # fuse into one windowed op per kw, straight from PSUM.  All
            # four stay on the vector engine: Tile serializes same-tile
            # writers across engines anyway, and a scalar-engine shift
            # would let unrelated scalar work interleave into the chain.
            nc.vector.tensor_scalar_max(
                imkw[0:32, 1:17, 1:8], p_z1[:, :, 1:15:2], 0.0)
            nc.vector.tensor_scalar_max(
                imkw[32:64, 1:17, 0:8], p_z1[:, :, 0:16:2], 0.0)
            nc.vector.tensor_scalar_max(
                imkw[64:96, 1:17, 0:8], p_z1[:, :, 1:16:2], 0.0)
            nc.vector.tensor_scalar_max(
                imkw[96:128, 1:17, 0:7], p_z1[:, :, 2:16:2], 0.0)

            # ---- conv2: 4 accumulating matmuls (one per kh) ----
            p_z2 = psA.tile([64, 64], F32, tag="z2")
            for kh in range(4):
                nc.tensor.matmul(
                    p_z2[:],
                    sb_w2[:, kh, :],
                    imkw[:, kh:min(kh + 16, 18):2, :],
                    start=(kh == 0), stop=(kh == 3),
                )

            # ---- Wvo = Wv @ Wo; PE is otherwise idle while zq is built.
            p_wvo = psA.tile([64, 64], F32, tag="wvo")
            nc.tensor.matmul(p_wvo[:], sb_tail[0:64, 0:64],
                             sb_tail[0:64, 64:128], start=True, stop=True)

            # zq relu on the scalar engine: shorter from PSUM and keeps
            # the vector queue free for the imkw shifts.
            sb_zq = work.tile([64, 64], F32R, tag="zq")
            nc.scalar.activation(sb_zq[:], p_z2[:], RELU)

            # wvo65 = [Wvo | e_Z]: border (zeros + lone 1 at [64,64]) came
            # from the host inside tail; the copy fills the Wvo block.
            sb_wvo65 = sb_tail[0:65, 128:194]
            nc.scalar.copy(sb_wvo65[0:64, 0:64], p_wvo[:])

            # ---- scoresT: (mem128, pos) chunks; lkT chunk stationary.
            # Chunks alternate between two PSUM tiles (= two banks) so
            # exp of chunk c can run while the PE writes chunk c+1: the
            # exp is split four ways and each G matmul chases its own
            # exp chunk.
            p_sT0 = psA.tile([128, 2, 64], F32, tag="sT0")
            p_sT1 = psA.tile([128, 2, 64], F32, tag="sT1")
            sT = lambda c: (p_sT0 if c % 2 == 0 else p_sT1)[:, c // 2, :]
            for c in range(4):
                nc.tensor.matmul(
                    sT(c),
                    sb_lkT[:, 128 * c:128 * (c + 1)], sb_zq[:],
                    start=True, stop=True,
                )

            # unnormalized softmax: E = exp(s/8).  |s/8| << 1 here, so
            # max-subtraction is unnecessary in fp32.
            sb_E0 = work.tile([128, 2, 64], F32R, tag="E0")
            sb_E1 = work.tile([128, 2, 64], F32R, tag="E1")
            E = lambda c: (sb_E0 if c % 2 == 0 else sb_E1)[:, c // 2, :]
            for c in range(4):
                nc.scalar.activation(E(c), sT(c), EXP, scale=0.125)

            # ---- [G; Z][d, pos] = sum_m [lk | 1][m, d] * E[m, pos] ----
            p_g = psA.tile([65, 64], F32, tag="g")
            for c in range(4):
                nc.tensor.matmul(
                    p_g[:], sb_lko[:, c, :], E(c),
                    start=(c == 0), stop=(c == 3),
                )
            sb_g = work.tile([65, 64], F32R, tag="gr")
            nc.vector.tensor_copy(sb_g[:], p_g[:])

            # ---- p_o[pos, 0:64] = (G.T @ Wvo)[pos, ch']; p_o[pos, 64] = Z
            p_o = psA.tile([64, 66], F32, tag="o")
            nc.tensor.matmul(p_o[:], sb_g[:], sb_wvo65,
                             start=True, stop=True)
            sb_rz = work.tile([64, 1], F32, tag="rz")
            nc.vector.reciprocal(sb_rz[:], p_o[:, 64:65])

            # scale + store in two halves: vector scales half 1 while the
            # scalar engine scales half 2 (Copy activation with a
            # per-partition scale), each feeding its own DMA queue.  Two
            # separate tiles — a shared tile would make Tile serialize
            # the cross-engine writers.
            sb_oa = work.tile([32, 64], F32, tag="oa")
            sb_ob = work.tile([64, 64], F32, tag="ob")
            nc.vector.tensor_scalar_mul(
                sb_oa[:, :], p_o[0:32, 0:64], sb_rz[0:32, :])
            nc.sync.dma_start(out_d.ap()[:32, :], sb_oa[:, :])
            nc.scalar.activation(
                sb_ob[32:64, :], p_o[32:64, 0:64],
                mybir.ActivationFunctionType.Copy, scale=sb_rz[32:64, :])
            nc.scalar.dma_start(out_d.ap()[32:, :], sb_ob[32:64, :])

    nc.compile()
    return nc


def _build_raw(wait_out: bool = True):
    """No-bias variant, raw bass (no TileContext): manual semaphores,
    no Tile entry/exit barriers or semaphore range-clear.  Same math and
    schedule as _build_fast, hand-synchronized.

    PSUM banks: z1 | z2 | wvo | sT0 | sT1 | g | o  (7 of 8).
    scoresT chunks alternate sT0/sT1 so exp(c) [ACT read] never touches
    the bank the PE is writing (c+1); the PE re-enters a bank only after
    waiting for the exp that read it.
    """
    from contextlib import ExitStack

    nc = bacc.Bacc("TRN2", target_bir_lowering=False, debug=False,
                   enable_asserts=False)

    xw_d = nc.dram_tensor("xw", [48, 288], F32R, kind="ExternalInput")
    w2k_d = nc.dram_tensor("w2k", [128, 4, 64], F32R, kind="ExternalInput")
    lkT_d = nc.dram_tensor("lkT", [64, 512], F32R, kind="ExternalInput")
    lko_d = nc.dram_tensor("lko", [128, 4, 65], F32R, kind="ExternalInput")
    tail_d = nc.dram_tensor("tail", [65, 256], F32R, kind="ExternalInput")
    out_d = nc.dram_tensor("out", [64, 64], F32, kind="ExternalOutput")

    es = ExitStack()
    sb_xw = es.enter_context(nc.sbuf_tensor([48, 288], F32R))
    sb_w2 = es.enter_context(nc.sbuf_tensor([128, 4, 64], F32R))
    sb_tail = es.enter_context(nc.sbuf_tensor([65, 256], F32R))
    sb_lkT = es.enter_context(nc.sbuf_tensor([64, 512], F32R))
    sb_lko = es.enter_context(nc.sbuf_tensor([128, 4, 65], F32R))
    sb_zero = es.enter_context(nc.sbuf_tensor([128, 18, 8], F32))
    imkw = es.enter_context(nc.sbuf_tensor([128, 18, 8], F32R))
    sb_zq = es.enter_context(nc.sbuf_tensor([64, 64], F32R))
    sb_E0 = es.enter_context(nc.sbuf_tensor([128, 2, 64], F32R))
    sb_E1 = es.enter_context(nc.sbuf_tensor([128, 2, 64], F32R))
    sb_g = es.enter_context(nc.sbuf_tensor([65, 64], F32R))
    sb_rz = es.enter_context(nc.sbuf_tensor([64, 1], F32))
    sb_oa = es.enter_context(nc.sbuf_tensor([32, 64], F32))
    sb_ob = es.enter_context(nc.sbuf_tensor([64, 64], F32))

    p_z1 = es.enter_context(nc.psum_tensor([32, 16, 16], F32))
    p_z2 = es.enter_context(nc.psum_tensor([64, 64], F32))
    p_wvo = es.enter_context(nc.psum_tensor([64, 64], F32))
    p_sT0 = es.enter_context(nc.psum_tensor([128, 2, 64], F32))
    p_sT1 = es.enter_context(nc.psum_tensor([128, 2, 64], F32))
    p_g = es.enter_context(nc.psum_tensor([65, 64], F32))
    p_o = es.enter_context(nc.psum_tensor([64, 66], F32))

    sXW = es.enter_context(nc.semaphore("sXW"))
    sTL = es.enter_context(nc.semaphore("sTL"))
    sLK = es.enter_context(nc.semaphore("sLK"))
    sW2 = es.enter_context(nc.semaphore("sW2"))
    sLO = es.enter_context(nc.semaphore("sLO"))
    sPE = es.enter_context(nc.semaphore("sPE"))
    sA = es.enter_context(nc.semaphore("sA"))
    sV = es.enter_context(nc.semaphore("sV"))
    sO1 = es.enter_context(nc.semaphore("sO1"))
    sO2 = es.enter_context(nc.semaphore("sO2"))

    sb_wvo65 = sb_tail[0:65, 128:194]
    sT = lambda c: (p_sT0 if c % 2 == 0 else p_sT1)[:, c // 2, :]
    E = lambda c: (sb_E0 if c % 2 == 0 else sb_E1)[:, c // 2, :]

    # ---- sync: three input DMAs, out half 1, final completion waits
    nc.sync.dma_start(sb_xw[:], xw_d.ap()).then_inc(sXW, 16)
    nc.sync.dma_start(sb_tail[:], tail_d.ap()).then_inc(sTL, 16)
    nc.sync.dma_start(sb_lkT[:], lkT_d.ap()).then_inc(sLK, 16)
    nc.sync.wait_ge(sV, 9)
    nc.sync.dma_start(out_d.ap()[:32, :], sb_oa[:]).then_inc(sO1, 16)
    if wait_out:
        nc.sync.wait_ge(sO1, 16)
        nc.sync.wait_ge(sO2, 16)

    # ---- gpsimd: lko DMA only
    nc.gpsimd.dma_start(sb_lko[:], lko_d.ap()).then_inc(sLO, 16)

    # ---- vector: zero prep, shifts, g cast, recip, out half 1
    # Under relaxed ordering, same-engine program order is NOT a data
    # barrier: a later op's reads/writes can overlap an earlier op's
    # in-flight writes.  Engines issue strict-FIFO with in-order
    # completion, so a blocking wait also orders everything emitted
    # after it, and counting sems stay valid.  Chains below cover the
    # true same-engine hazards only: memset->cast (RAW on sb_zero),
    # cast->shifts (WAW on imkw, one wait covers all four), recip->mul1
    # (RAW on sb_rz).
    nc.vector.memset(sb_zero[:], 0.0).then_inc(sV)                    # 1
    nc.vector.wait_ge(sV, 1)
    nc.vector.tensor_copy(imkw[:], sb_zero[:]).then_inc(sV)           # 2
    nc.vector.wait_ge(sV, 2)
    nc.vector.wait_ge(sPE, 1)
    nc.vector.tensor_scalar_max(
        imkw[0:32, 1:17, 1:8], p_z1[:, :, 1:15:2], 0.0).then_inc(sV)  # 3
    nc.vector.tensor_scalar_max(
        imkw[32:64, 1:17, 0:8], p_z1[:, :, 0:16:2], 0.0).then_inc(sV)
    nc.vector.tensor_scalar_max(
        imkw[64:96, 1:17, 0:8], p_z1[:, :, 1:16:2], 0.0).then_inc(sV)
    nc.vector.tensor_scalar_max(
        imkw[96:128, 1:17, 0:7], p_z1[:, :, 2:16:2], 0.0).then_inc(sV)  # 6
    nc.vector.wait_ge(sPE, 14)
    nc.vector.tensor_copy(sb_g[:], p_g[:]).then_inc(sV)               # 7
    nc.vector.wait_ge(sPE, 15)
    nc.vector.reciprocal(sb_rz[:], p_o[:, 64:65]).then_inc(sV)        # 8
    nc.vector.wait_ge(sV, 8)
    nc.vector.tensor_scalar_mul(
        sb_oa[:], p_o[0:32, 0:64], sb_rz[0:32, :]).then_inc(sV)       # 9

    # ---- scalar: w2k DMA, wvo copy, zq relu, 4 exps, out half 2
    # No self-chains needed: the scalar ops touch disjoint tiles and
    # complete in issue order, so the sA counts other engines wait on
    # stay valid (see vector comment).
    nc.scalar.dma_start(sb_w2[:], w2k_d.ap()).then_inc(sW2, 16)
    nc.scalar.wait_ge(sPE, 2)
    nc.scalar.copy(sb_wvo65[0:64, 0:64], p_wvo[:]).then_inc(sA)       # 1
    nc.scalar.wait_ge(sPE, 6)
    nc.scalar.activation(sb_zq[:], p_z2[:], RELU).then_inc(sA)        # 2
    for c in range(4):
        nc.scalar.wait_ge(sPE, 7 + c)
        nc.scalar.activation(E(c), sT(c), EXP, scale=0.125).then_inc(sA)  # 3..6
    # mul2 serialized after mul1: V and A must not read PSUM bank 'o'
    # concurrently (same-bank V+A access is unarbitrated).
    nc.scalar.wait_ge(sV, 9)
    nc.scalar.activation(
        sb_ob[32:64, :], p_o[32:64, 0:64],
        mybir.ActivationFunctionType.Copy, scale=sb_rz[32:64, :]).then_inc(sA)
    # explicit wait: under relaxed ordering the engine dispatches the DMA
    # trigger before the preceding ACTIVATE's datapath drains, so without
    # a semaphore the HWDGE can read sb_ob mid-write.
    nc.scalar.wait_ge(sA, 7)
    nc.scalar.dma_start(out_d.ap()[32:, :], sb_ob[32:64, :]).then_inc(sO2, 16)

    # ---- tensor: conv1, wvo, conv2, scoresT, G, final
    nc.tensor.wait_ge(sXW, 16)
    nc.tensor.matmul(p_z1[:], sb_xw[:, 256:288], sb_xw[:, :256],
                     start=True, stop=True).then_inc(sPE)             # 1
    nc.tensor.wait_ge(sTL, 16)
    nc.tensor.matmul(p_wvo[:], sb_tail[0:64, 0:64], sb_tail[0:64, 64:128],
                     start=True, stop=True).then_inc(sPE)             # 2
    nc.tensor.wait_ge(sV, 6)
    nc.tensor.wait_ge(sW2, 16)
    for kh in range(4):
        nc.tensor.matmul(
            p_z2[:], sb_w2[:, kh, :], imkw[:, kh:min(kh + 16, 18):2, :],
            start=(kh == 0), stop=(kh == 3)).then_inc(sPE)            # 3..6
    nc.tensor.wait_ge(sA, 2)
    nc.tensor.wait_ge(sLK, 16)
    nc.tensor.matmul(sT(0), sb_lkT[:, 0:128], sb_zq[:],
                     start=True, stop=True).then_inc(sPE)             # 7
    nc.tensor.matmul(sT(1), sb_lkT[:, 128:256], sb_zq[:],
                     start=True, stop=True).then_inc(sPE)             # 8
    nc.tensor.wait_ge(sA, 3)  # exp(c0) released bank sT0
    nc.tensor.matmul(sT(2), sb_lkT[:, 256:384], sb_zq[:],
                     start=True, stop=True).then_inc(sPE)             # 9
    nc.tensor.wait_ge(sA, 4)  # exp(c1) released bank sT1
    nc.tensor.matmul(sT(3), sb_lkT[:, 384:512], sb_zq[:],
                     start=True, stop=True).then_inc(sPE)             # 10
    nc.tensor.wait_ge(sLO, 16)
    nc.tensor.matmul(p_g[:], sb_lko[:, 0, :], E(0),
                     start=True, stop=False).then_inc(sPE)            # 11
    nc.tensor.wait_ge(sA, 4)
    nc.tensor.matmul(p_g[:], sb_lko[:, 1, :], E(1),
                     start=False, stop=False).then_inc(sPE)           # 12
    nc.tensor.wait_ge(sA, 5)
    nc.tensor.matmul(p_g[:], sb_lko[:, 2, :], E(2),
                     start=False, stop=False).then_inc(sPE)           # 13
    nc.tensor.wait_ge(sA, 6)
    nc.tensor.matmul(p_g[:], sb_lko[:, 3, :], E(3),
                     start=False, stop=True).then_inc(sPE)            # 14
    nc.tensor.wait_ge(sV, 7)
    nc.tensor.matmul(p_o[:], sb_g[:], sb_wvo65,
                     start=True, stop=True).then_inc(sPE)             # 15

    es.close()
    nc.compile()
    return nc


def _build_bias():
    """General variant (nonzero biases): JVP via sign masks."""
    nc = bacc.Bacc("TRN2", target_bir_lowering=False, debug=False,
                   enable_asserts=False)

    x_im = nc.dram_tensor("x_im", [48, 256], F32R, kind="ExternalInput")
    w1r = nc.dram_tensor("w1r", [48, 32], F32R, kind="ExternalInput")
    w2k = nc.dram_tensor("w2k", [128, 4, 64], F32R, kind="ExternalInput")
    lkT = nc.dram_tensor("lkT", [64, 512], F32R, kind="ExternalInput")
    wvT = nc.dram_tensor("wvT", [64, 64], F32R, kind="ExternalInput")
    ident_d = nc.dram_tensor("ident", [64, 64], F32R, kind="ExternalInput")
    wo = nc.dram_tensor("wo", [64, 64], F32R, kind="ExternalInput")
    b1 = nc.dram_tensor("b1", [32, 1], F32, kind="ExternalInput")
    b2 = nc.dram_tensor("b2", [64, 1], F32, kind="ExternalInput")
    out_d = nc.dram_tensor("out", [64, 64], F32, kind="ExternalOutput")

    with tile.TileContext(nc) as tc:
        with (
            tc.tile_pool(name="consts", bufs=1) as consts,
            tc.tile_pool(name="work", bufs=1) as work,
            tc.tile_pool(name="psA", bufs=1, space="PSUM") as psA,
            tc.tile_pool(name="psT", bufs=2, space="PSUM") as psT,
        ):
            sb_xim = consts.tile([48, 256], F32R, tag="xim")
            nc.sync.dma_start(sb_xim[:24, :], x_im.ap()[:24, :])
            nc.scalar.dma_start(sb_xim[24:, :], x_im.ap()[24:, :])
            ident = consts.tile([64, 64], F32R, tag="ident")
            nc.gpsimd.dma_start(ident[:], ident_d.ap())
            sb_w1 = consts.tile([48, 32], F32R, tag="w1")
            nc.gpsimd.dma_start(sb_w1[:], w1r.ap())
            sb_w2 = consts.tile([128, 4, 64], F32R, tag="w2")
            nc.sync.dma_start(sb_w2[:, :2, :], w2k.ap()[:, :2, :])
            nc.scalar.dma_start(sb_w2[:, 2:, :], w2k.ap()[:, 2:, :])
            sb_lkT = consts.tile([64, 512], F32R, tag="lkT")
            nc.gpsimd.dma_start(sb_lkT[:, :256], lkT.ap()[:, :256])
            nc.sync.dma_start(sb_lkT[:, 256:], lkT.ap()[:, 256:])
            sb_wvT = consts.tile([64, 64], F32R, tag="wvT")
            nc.gpsimd.dma_start(sb_wvT[:], wvT.ap())
            sb_wo = consts.tile([64, 64], F32R, tag="wo")
            nc.scalar.dma_start(sb_wo[:], wo.ap())
            sb_b1 = consts.tile([32, 1], F32, tag="b1")
            nc.gpsimd.dma_start(sb_b1[:], b1.ap())
            sb_b2 = consts.tile([64, 1], F32, tag="b2")
            nc.gpsimd.dma_start(sb_b2[:], b2.ap())

            sb_zero = consts.tile([128, 18, 8], F32, tag="zero")
            nc.vector.memset(sb_zero[:], 0.0)
            sb_one = consts.tile([65, 2], F32R, tag="one")
            nc.vector.tensor_scalar_add(sb_one[64:65, :], sb_zero[64:65, 0, :2], 1.0)

            sb_lk = work.tile([128, 4, 65], F32R, tag="lk")
            nc.vector.tensor_scalar_add(sb_lk[:, :, 64:65],
                                        sb_zero[:, :4, :1], 1.0)

            # ---- conv1 ----
            p_z1 = psA.tile([32, 16, 16], F32, tag="a")
            nc.tensor.matmul(p_z1[:], sb_w1[:], sb_xim[:],
                             start=True, stop=True)

            def conv2(imkw, ps_tag):
                p = psA.tile([64, 64], F32, tag=ps_tag)
                for kh in range(4):
                    nc.tensor.matmul(
                        p[:],
                        sb_w2[:, kh, :],
                        imkw[:, kh:min(kh + 16, 18):2, :],
                        start=(kh == 0), stop=(kh == 3),
                    )
                return p

            # a1 = relu(z1 + b1); t1m = z1 * sign(a1)
            sb_a1 = work.tile([32, 16, 16], F32, tag="a1")
            nc.scalar.activation(
                sb_a1[:], p_z1[:], RELU, bias=sb_b1[:], scale=1.0)
            sb_m1 = work.tile([32, 16, 16], F32, tag="m1")
            nc.scalar.activation(
                sb_m1[:], sb_a1[:], mybir.ActivationFunctionType.Sign)
            sb_t1 = work.tile([32, 16, 16], F32, tag="t1")
            nc.vector.tensor_mul(sb_t1[:], p_z1[:], sb_m1[:])

            def shifts(dst, src):
                nc.vector.tensor_copy(dst[0:32, 1:17, 1:8], src[:, :, 1:15:2])
                nc.vector.tensor_copy(dst[32:64, 1:17, 0:8], src[:, :, 0:16:2])
                nc.vector.tensor_copy(dst[64:96, 1:17, 0:8], src[:, :, 1:16:2])
                nc.vector.tensor_copy(dst[96:128, 1:17, 0:7], src[:, :, 2:16:2])

            imkw = work.tile([128, 18, 8], F32R, tag="imkw")
            nc.vector.tensor_copy(imkw[:], sb_zero[:])
            shifts(imkw, sb_a1)
            p_z2 = conv2(imkw, "b")
            imkw2 = work.tile([128, 18, 8], F32R, tag="imkw2")
            nc.vector.tensor_copy(imkw2[:], sb_zero[:])
            shifts(imkw2, sb_t1)
            p_t2 = conv2(imkw2, "e")

            sb_zq = work.tile([64, 64], F32R, tag="zq")
            sb_z2r = work.tile([64, 64], F32, tag="z2r")
            nc.scalar.activation(
                sb_z2r[:], p_z2[:], RELU, bias=sb_b2[:], scale=1.0)
            sb_m2 = work.tile([64, 64], F32, tag="m2")
            nc.scalar.activation(
                sb_m2[:], sb_z2r[:], mybir.ActivationFunctionType.Sign)
            nc.vector.tensor_mul(sb_zq[:], p_t2[:], sb_m2[:])

            # ---- scoresT + lookup transposes ----
            p_sT = psA.tile([128, 4, 64], F32, tag="c")
            for c in range(4):
                nc.tensor.matmul(
                    p_sT[:, c, :],
                    sb_lkT[:, 128 * c:128 * (c + 1)], sb_zq[:],
                    start=True, stop=True,
                )
            for c in range(4):
                p_lk = psT.tile([128, 64], F32, tag="ptr")
                nc.tensor.matmul(
                    p_lk[:], sb_lkT[:, 128 * c:128 * (c + 1)], ident[:],
                    start=True, stop=True,
                )
                nc.scalar.copy(sb_lk[:, c, :64], p_lk[:])

            p_wvo = psA.tile([64, 64], F32, tag="d")
            nc.tensor.matmul(p_wvo[:], sb_wvT[:], sb_wo[:],
                             start=True, stop=True)
            sb_wvo = work.tile([64, 64], F32R, tag="wvo")
            nc.scalar.copy(sb_wvo[:], p_wvo[:])

            sb_E = work.tile([128, 4, 64], F32R, tag="E")
            nc.scalar.activation(sb_E[:], p_sT[:], EXP, scale=0.125)

            p_g = psA.tile([65, 64], F32, tag="d")
            for c in range(4):
                nc.tensor.matmul(
                    p_g[:], sb_lk[:, c, :], sb_E[:, c, :],
                    start=(c == 0), stop=(c == 3),
                )
            sb_g = work.tile([65, 64], F32R, tag="g")
            nc.vector.tensor_copy(sb_g[:], p_g[:])

            p_zT = psA.tile([64, 2], F32, tag="b")
            nc.tensor.matmul(p_zT[:], sb_g[64:65, :].bitcast(F32),
                             sb_one[64:65, :].bitcast(F32),
                             start=True, stop=True)
            sb_rz = work.tile([64, 1], F32, tag="rz")
            nc.vector.reciprocal(sb_rz[:], p_zT[:, :1])

            p_o = psA.tile([64, 64], F32, tag="a")
            nc.tensor.matmul(p_o[:], sb_g[:64, :], sb_wvo[:],
                             start=True, stop=True)
            sb_out = work.tile([64, 64], F32, tag="out")
            nc.vector.tensor_scalar_mul(sb_out[:], p_o[:], sb_rz[:])
            nc.sync.dma_start(out_d.ap()[:32, :], sb_out[:32, :])
            nc.scalar.dma_start(out_d.ap()[32:, :], sb_out[32:, :])

    nc.compile()
    return nc


def _get_nc(with_bias: bool):
    if with_bias not in _COMPILED:
        if with_bias:
            nc = _build_bias()
        elif os.environ.get("KERNEL_RAW"):
            nc = _build_raw(wait_out=not os.environ.get("KERNEL_NOWAIT"))
        else:
            nc = _build_fast()
        _COMPILED[with_bias] = nc
    return _COMPILED[with_bias]


def _im2col(x):
    """(B, 3, 32, 32) -> (B, 48, 256) im2col for conv1 (layout only)."""
    xp = np.zeros((B, CIN, 34, 34), np.float32)
    xp[:, :, 1:33, 1:33] = x
    xim = np.empty((B, CIN, 4, 4, 16, 16), np.float32)
    for kh in range(4):
        for kw in range(4):
            xim[:, :, kh, kw] = xp[:, :, kh:kh + 32:2, kw:kw + 32:2]
    return np.ascontiguousarray(xim.reshape(B, 48, 256))


def kernel(x, conv1_w, conv1_b, conv2_w, conv2_b, lookup, Wv, Wo):
    global last_exec_time_ns, last_trace_path
    x = np.asarray(x, np.float32)
    w1 = np.asarray(conv1_w, np.float32)
    b1 = np.asarray(conv1_b, np.float32)
    w2 = np.asarray(conv2_w, np.float32)
    b2 = np.asarray(conv2_b, np.float32)
    lk = np.ascontiguousarray(np.asarray(lookup, np.float32))
    wv = np.ascontiguousarray(np.asarray(Wv, np.float32))
    wo = np.ascontiguousarray(np.asarray(Wo, np.float32))

    with_bias = bool(np.any(b1 != 0.0) or np.any(b2 != 0.0))

    # host-side layout prep (no arithmetic): im2col of padded x, weight
    # transposes/re-chunking to the matmul-native layouts.
    xim = _im2col(x)
    w1r = np.ascontiguousarray(w1.transpose(1, 2, 3, 0).reshape(48, 32))
    # w2k[(kw*32+ci), kh, co] = w2[co, ci, kh, kw]
    w2k = np.ascontiguousarray(w2.transpose(3, 1, 2, 0).reshape(128, 4, 64))
    lkT = np.ascontiguousarray(lk.T)
    wvT = np.ascontiguousarray(wv.T)

    if not with_bias:
        # xw = [im2col | w1r]
        xw = np.concatenate([xim, np.broadcast_to(w1r, (B, 48, 32))], axis=2)
        xw = np.ascontiguousarray(xw)
        # lko[p, c, :64] = lookup[128c+p, :]; lko[p, c, 64] = 1
        lko = np.ones((128, 4, 65), np.float32)
        lko[:, :, :64] = lk.reshape(4, 128, C2).transpose(1, 0, 2)
        # tail = [wvT | wo | wvo65-border]: border is zeros with a lone 1
        # at [64, 64] of the 65x65 block (the Z-passthrough column).
        tail = np.zeros((65, 256), np.float32)
        tail[0:64, 0:64] = wvT
        tail[0:64, 64:128] = wo
        tail[64, 192] = 1.0
        shared = {"w2k": w2k, "lkT": lkT, "lko": lko, "tail": tail}
        in_maps = [dict(shared, xw=xw[c % B]) for c in range(N_CORES)]
    else:
        shared = {"w1r": w1r, "w2k": w2k, "lkT": lkT, "wvT": wvT, "wo": wo,
                  "ident": np.eye(64, dtype=np.float32),
                  "b1": np.ascontiguousarray(b1.reshape(32, 1)),
                  "b2": np.ascontiguousarray(b2.reshape(64, 1))}
        in_maps = [dict(shared, x_im=xim[c % B]) for c in range(N_CORES)]

    nc = _get_nc(with_bias)
    trace = bool(os.environ.get("KERNEL_TRACE"))
    res = run_bass_kernel_spmd(
        nc, in_maps, core_ids=list(range(N_CORES)),
        trace=trace, trace_cores=[0] if trace else None,
    )
    last_exec_time_ns = res.exec_time_ns
    if res.instructions_and_trace:
        last_trace_path = res.instructions_and_trace[1]

    # device emits (pos, ch') per sample; host transposes (layout only)
    out = np.stack([res.results[b]["out"].T for b in range(B)])
    return np.ascontiguousarray(out.reshape(B, C2, 8, 8))
